# revision 23
# baseline (speedup 1.0000x reference)
"""Gated Slot Attention (GSA) Trainium2 kernel, v3.

Sharding: B*H = 8 lanes -> 8 cores (core = b*4 + h). Kernel 1 ("gsa") does the
per-lane projections + chunked two-pass GLA recurrence, emitting z = silu(o)
feature-major [DV, T] plus the per-lane sum-of-squares row (for RMSNorm).
The host reduces the sq rows across the 4 head-lanes of each batch and
computes the rsqrt denominators; kernel 2 ("final") is then a pure
GEMM + per-row scale with rows of (b,t) split across cores.

Kernel-1 structure:
  Phase A  - weight-stationary projections over the full T=2048: f psums +
             e1=exp(-x) first, then the gate chain (ln/exp on Act, s/scan/
             rlam/st on DVE) emitted BEFORE the q,k,v matmuls so it overlaps
             their PE time; q,k,v use a single fused Silu activation each.
  Sweeps   - 8 chunks of C=256 in a 4-stage software pipeline: at iteration
             i the kernel emits softmax-prep for chunk i-1, ptm for chunk
             i+1, p2m for i-1, transposes for i+1, the et/Hk core for i and
             the zt/Hv core for i-1.  Every PE stage therefore depends only
             on work emitted >= 1 iteration earlier.
             Triangular masking only on the diagonal 128-blocks; the
             strictly-lower block is never computed.
             Hk' = Lend*(Hk + k^T St), Hv' = Lend*(Hv + St^T v) (St-form),
             softmax 1/colsum folded into the final o-multiply.
"""
import sys
sys.path.insert(0, '/opt/trn_rl_repo')

import numpy as np
import ml_dtypes

import concourse.bass as bass
import concourse.bacc as bacc
import concourse.tile as tile
import concourse.mybir as mybir
import concourse.bass_utils as bass_utils

BF = mybir.dt.bfloat16
F32 = mybir.dt.float32
AF = mybir.ActivationFunctionType
OP = mybir.AluOpType

B, T, D = 2, 2048, 1024
H, DK, DV, M = 4, 256, 256, 256
C = 256            # chunk length
NCHUNK = T // C
GATE_NORM = 8.0
EPS = 1e-5

_cache = {}


def build_gsa():
    """Kernel 1: per-lane projections + chunked GLA, block-merged.
    T is processed in 4 blocks of 512 (2 chunks each): each block's
    projections (64 N=512 matmuls) are followed by 2 sweep iterations whose
    cross-engine chains hide inside the next block's PE stream.
    Outputs zT [256, 2048] bf16 and sq [1, 2048] f32 (sum over dv of z^2)."""
    nc = bacc.Bacc("TRN2", target_bir_lowering=False, debug=False, num_devices=8)
    hsT_d = nc.dram_tensor("hst", [D, T], BF, kind="ExternalInput").ap()
    # wall columns: f 0:256 | q 256:512 | k 512:768 | v 768:1024
    w_d = nc.dram_tensor("wall", [D, 1024], BF, kind="ExternalInput").ap()
    mask_d = nc.dram_tensor("mask", [128, 128], BF, kind="ExternalInput").ap()
    ident_d = nc.dram_tensor("ident", [128, 128], BF, kind="ExternalInput").ap()
    ident32_d = nc.dram_tensor("ident32", [128, 128], F32, kind="ExternalInput").ap()
    z_d = nc.dram_tensor("z", [DV, T], BF, kind="ExternalOutput").ap()
    sq_d = nc.dram_tensor("sq", [1, T], F32, kind="ExternalOutput").ap()

    hsv = hsT_d.rearrange("(a p) t -> p a t", p=128)
    wv = w_d.rearrange("(a p) o -> p a o", p=128)
    zv = z_d.rearrange("(a p) t -> p a t", p=128)

    with tile.TileContext(nc) as tc:
        with (
            tc.tile_pool(name="persist", bufs=1) as pp,
            tc.tile_pool(name="work", bufs=2) as wp,
            tc.tile_pool(name="aw", bufs=2) as aw,
            tc.tile_pool(name="psA", bufs=3, space="PSUM") as psA,
            tc.tile_pool(name="psm", bufs=5, space="PSUM") as psmp,
        ):
            hs = pp.tile([128, 8, T], BF, tag="hs")
            w = pp.tile([128, 8, 1024], BF, tag="w")
            msk = pp.tile([128, 128], BF, tag="msk")
            ident = pp.tile([128, 128], BF, tag="ident")
            ident32 = pp.tile([128, 128], F32, tag="ident32")
            ones_col = pp.tile([128, 1], BF, tag="onescol")
            qT = pp.tile([128, 2, T], BF, tag="qT")
            kT = pp.tile([128, 2, T], BF, tag="kT")
            vT = pp.tile([128, 2, T], BF, tag="vT")
            st = pp.tile([128, 2, T], BF, tag="st")
            lam = pp.tile([128, 2, T], F32, tag="lam")
            o = pp.tile([128, 2, T], BF, tag="o")
            z = pp.tile([128, 2, T], BF, tag="z")
            sqrow = pp.tile([1, T], F32, tag="sqrow")
            hkb0 = pp.tile([128, 2, 256], BF, tag="hkb0")
            hkb1 = pp.tile([128, 2, 256], BF, tag="hkb1")
            hvb0 = pp.tile([128, 2, 256], BF, tag="hvb0")
            hvb1 = pp.tile([128, 2, 256], BF, tag="hvb1")
            hkbs, hvbs = [hkb0, hkb1], [hvb0, hvb1]

            # DMA order: what block 0 needs first, then the rest.
            nc.sync.dma_start(out=w[:, :, 0:256], in_=wv[:, :, 0:256])
            nc.sync.dma_start(out=hs[:, :, 0:512], in_=hsv[:, :, 0:512])
            nc.sync.dma_start(out=w[:, :, 256:1024], in_=wv[:, :, 256:1024])
            for tq in range(1, 4):
                nc.sync.dma_start(out=hs[:, :, tq * 512:(tq + 1) * 512],
                                  in_=hsv[:, :, tq * 512:(tq + 1) * 512])
            nc.sync.dma_start(out=msk, in_=mask_d)
            nc.sync.dma_start(out=ident, in_=ident_d)
            nc.sync.dma_start(out=ident32, in_=ident32_d)
            nc.vector.memset(ones_col, 1.0)
            nc.vector.memset(hkb1, 0.0)
            nc.vector.memset(hvb1, 0.0)

            QKV_COLS = [(qT, 0, 256), (qT, 1, 384), (kT, 0, 512), (kT, 1, 640),
                        (vT, 0, 768), (vT, 1, 896)]

            def proj_block(blk):
                """Projections + gate math for T-block blk (chunks 2b, 2b+1)."""
                t0 = blk * 512
                e1 = aw.tile([128, 2, 512], BF, tag="aw")
                for ot in range(2):
                    ps = psA.tile([128, 512], F32, tag="ps", name="psf")
                    for dt in range(8):
                        nc.tensor.matmul(ps, lhsT=w[:, dt, ot * 128:(ot + 1) * 128],
                                         rhs=hs[:, dt, t0:t0 + 512],
                                         start=(dt == 0), stop=(dt == 7))
                    nc.scalar.activation(e1[:, ot, :], ps, AF.Exp, scale=-1.0)
                nsp = aw.tile([128, 2, 512], BF, tag="aw")
                gg = aw.tile([128, 2, 512], F32, tag="awg")
                ss2 = aw.tile([128, 2, 512], BF, tag="aws")
                rl = aw.tile([128, 2, 512], BF, tag="awr")
                nc.scalar.activation(nsp, e1, AF.Ln, bias=1.0)
                nc.scalar.activation(gg, nsp, AF.Exp, scale=-1.0 / GATE_NORM)
                nc.vector.tensor_scalar(out=ss2, in0=gg, scalar1=-1.0,
                                        scalar2=1.0, op0=OP.mult, op1=OP.add)
                for mt in range(2):
                    for ch in range(2):
                        nc.vector.tensor_tensor_scan(
                            lam[:, mt, t0 + ch * C:t0 + (ch + 1) * C],
                            gg[:, mt, ch * C:(ch + 1) * C],
                            gg[:, mt, ch * C:(ch + 1) * C], 1.0, OP.mult, OP.bypass)
                with nc.allow_low_precision(reason="1/lam bf16 ok (2e-2 tol)"):
                    nc.vector.reciprocal(rl, lam[:, :, t0:t0 + 512])
                nc.vector.tensor_tensor(st[:, :, t0:t0 + 512], ss2, rl, op=OP.mult)
                for dest, ot, col in QKV_COLS:
                    ps = psA.tile([128, 512], F32, tag="ps", name="psp")
                    for dt in range(8):
                        nc.tensor.matmul(ps, lhsT=w[:, dt, col:col + 128],
                                         rhs=hs[:, dt, t0:t0 + 512],
                                         start=(dt == 0), stop=(dt == 7))
                    nc.scalar.activation(dest[:, ot, t0:t0 + 512], ps, AF.Silu)

            st_tms, k_tms, v_tms, ptms, ets, bcls, qt2s, rbcs = \
                {}, {}, {}, {}, {}, {}, {}, {}

            def qt2_prep(cc):
                """Qt = lam * et for chunk cc (dep: et, 1 iter old)."""
                base = cc * C
                et = ets[cc]
                qt2 = wp.tile([128, 2, 256], BF, tag="qt2")
                for mt in range(2):
                    nc.gpsimd.tensor_tensor(qt2[:, mt, :], et[:, mt, :],
                                            lam[:, mt, base:base + C],
                                            op=OP.mult)
                qt2s[cc] = qt2

            def softmax_prep(cc):
                """cs, recip, rbc for chunk cc."""
                et = ets[cc]
                pmisc = psmp.tile([128, 512], F32, tag="pm")
                for mt in range(2):
                    nc.tensor.matmul(pmisc[0:1, 0:256], lhsT=ones_col,
                                     rhs=et[:, mt, :],
                                     start=(mt == 0), stop=(mt == 1))
                rrow = wp.tile([1, 256], F32, tag="rrow")
                nc.vector.reciprocal(rrow, pmisc[0:1, 0:256])
                rbc = wp.tile([128, 256], F32, tag="rbc")
                nc.gpsimd.partition_broadcast(rbc, rrow)
                rbcs[cc] = rbc

            def ptm_stage(c):
                """ptm = mask(k^T q) for chunk c (needs projections only)."""
                base = c * C
                pp0 = psmp.tile([128, 512], F32, tag="pm")
                for k2 in range(2):
                    nc.tensor.matmul(pp0[:, 0:256],
                                     lhsT=kT[:, k2, base:base + 128],
                                     rhs=qT[:, k2, base:base + 256],
                                     start=(k2 == 0), stop=(k2 == 1))
                for k2 in range(2):
                    nc.tensor.matmul(pp0[:, 256:384],
                                     lhsT=kT[:, k2, base + 128:base + 256],
                                     rhs=qT[:, k2, base + 128:base + 256],
                                     start=(k2 == 0), stop=(k2 == 1))
                ptm = wp.tile([128, 2, 256], BF, tag="ptm")
                nc.vector.tensor_tensor(ptm[:, 0, 0:128], pp0[:, 0:128],
                                        msk, op=OP.mult)
                nc.scalar.activation(ptm[:, 0, 128:256], pp0[:, 128:256], AF.Copy)
                nc.vector.tensor_tensor(ptm[:, 1, 128:256], pp0[:, 256:384],
                                        msk, op=OP.mult)
                ptms[c] = ptm

            def p2m_stage(cc):
                """p2m = mask(St^T Qt) for chunk cc."""
                base = cc * C
                qt2 = qt2s[cc]
                pp2 = psmp.tile([128, 512], F32, tag="pm")
                for mt in range(2):
                    nc.tensor.matmul(pp2[:, 0:256],
                                     lhsT=st[:, mt, base:base + 128],
                                     rhs=qt2[:, mt, :],
                                     start=(mt == 0), stop=(mt == 1))
                for mt in range(2):
                    nc.tensor.matmul(pp2[:, 256:384],
                                     lhsT=st[:, mt, base + 128:base + 256],
                                     rhs=qt2[:, mt, 128:256],
                                     start=(mt == 0), stop=(mt == 1))
                p2m = wp.tile([128, 2, 256], BF, tag="p2m")
                nc.vector.tensor_tensor(p2m[:, 0, 0:128], pp2[:, 0:128],
                                        msk, op=OP.mult)
                nc.scalar.activation(p2m[:, 0, 128:256], pp2[:, 128:256], AF.Copy)
                nc.vector.tensor_tensor(p2m[:, 1, 128:256], pp2[:, 256:384],
                                        msk, op=OP.mult)
                return p2m

            def transpose_stage(c):
                """st_tm, k_tm, v_tm via DMA-engine transposes; bcl."""
                base = c * C
                st_tm = wp.tile([128, 2, 256], BF, tag="sttm", bufs=3)
                k_tm = wp.tile([128, 2, 256], BF, tag="ktm")
                v_tm = wp.tile([128, 2, 256], BF, tag="vtm", bufs=3)
                for blk in range(2):
                    sl = slice(blk * 128, (blk + 1) * 128)
                    nc.sync.dma_start_transpose(st_tm[:, :, sl],
                                                st[:, blk, base:base + 256])
                    nc.sync.dma_start_transpose(k_tm[:, :, sl],
                                                kT[:, blk, base:base + 256])
                    nc.sync.dma_start_transpose(v_tm[:, :, sl],
                                                vT[:, blk, base:base + 256])
                st_tms[c], k_tms[c], v_tms[c] = st_tm, k_tm, v_tm

                # lend row -> bcl broadcast [128, 256] (Lend[m] on free dim)
                pmisc = psmp.tile([128, 512], F32, tag="pm")
                for mt in range(2):
                    nc.tensor.transpose(
                        pmisc[0:1, mt * 128:(mt + 1) * 128],
                        lam[:, mt, base + C - 1:base + C], ident32)
                lrow = wp.tile([1, 256], F32, tag="lrow")
                nc.vector.tensor_copy(lrow, pmisc[0:1, 0:256])
                bcl = wp.tile([128, 256], F32, tag="bcl")
                for mt in range(2):
                    nc.gpsimd.partition_broadcast(
                        bcl[:, mt * 128:(mt + 1) * 128],
                        lrow[0:1, mt * 128:(mt + 1) * 128])
                bcls[c] = bcl

            def hk_stage(c):
                """Hk' = Lend * (Hk + k^T St); all deps >= 1 iter old."""
                st_tm, k_tm = st_tms[c], k_tms.pop(c)
                bcl = bcls.pop(c)
                src_h, dst_h = hkbs[(c - 1) % 2], hkbs[c % 2]
                ph = psmp.tile([128, 512], F32, tag="pm")
                for kb in range(2):
                    for tb in range(2):
                        nc.tensor.matmul(ph[:, kb * 256:kb * 256 + 256],
                                         lhsT=k_tm[:, tb, kb * 128:(kb + 1) * 128],
                                         rhs=st_tm[:, tb, :],
                                         start=(tb == 0), stop=False)
                    nc.tensor.matmul(ph[:, kb * 256:kb * 256 + 256],
                                     lhsT=ident, rhs=src_h[:, kb, :],
                                     start=False, stop=True)
                for kb in range(2):
                    nc.vector.tensor_tensor(dst_h[:, kb, :],
                                            ph[:, kb * 256:kb * 256 + 256],
                                            bcl, op=OP.mult)

            def hv_stage(c):
                """Hv' = Lend[m] * (Hv + St^T v); all deps >= 1 iter old."""
                base = c * C
                st_tm, v_tm = st_tms[c], v_tms[c]
                src_h, dst_h = hvbs[(c - 1) % 2], hvbs[c % 2]
                phv = psmp.tile([128, 512], F32, tag="pm")
                for mt in range(2):
                    for tb in range(2):
                        nc.tensor.matmul(phv[:, mt * 256:mt * 256 + 256],
                                         lhsT=st_tm[:, tb, mt * 128:(mt + 1) * 128],
                                         rhs=v_tm[:, tb, :],
                                         start=(tb == 0), stop=False)
                    nc.tensor.matmul(phv[:, mt * 256:mt * 256 + 256],
                                     lhsT=ident, rhs=src_h[:, mt, :],
                                     start=False, stop=True)
                for mt in range(2):
                    nc.scalar.activation(dst_h[:, mt, :],
                                         phv[:, mt * 256:mt * 256 + 256],
                                         AF.Copy,
                                         scale=lam[:, mt, base + C - 1:base + C])

            def pass1_core(c):
                """etAB + ok + exp for chunk c."""
                base = c * C
                st_tm = st_tms[c]
                hkb = hkbs[(c - 1) % 2]
                ptm = ptms.pop(c)
                pe0 = psmp.tile([128, 512], F32, tag="pm")
                for mt in range(2):
                    nc.tensor.matmul(pe0[:, mt * 256:mt * 256 + 256],
                                     lhsT=st_tm[:, 0, mt * 128:(mt + 1) * 128],
                                     rhs=ptm[:, 0, :], start=True, stop=False)
                    nc.tensor.matmul(pe0[:, mt * 256 + 128:mt * 256 + 256],
                                     lhsT=st_tm[:, 1, mt * 128:(mt + 1) * 128],
                                     rhs=ptm[:, 1, 128:256], start=False, stop=False)
                    for k2 in range(2):
                        nc.tensor.matmul(pe0[:, mt * 256:mt * 256 + 256],
                                         lhsT=hkb[:, k2, mt * 128:(mt + 1) * 128],
                                         rhs=qT[:, k2, base:base + 256],
                                         start=False, stop=(k2 == 1))
                okl = wp.tile([128, 2, 256], F32, tag="okl")
                nc.vector.tensor_tensor(okl, pe0,
                                        lam[:, :, base:base + C], op=OP.mult)
                et = wp.tile([128, 2, 256], BF, tag="et")
                nc.scalar.activation(et, okl, AF.Exp)
                ets[c] = et

            def pass2_core(cc, p2m):
                """zt + o for chunk cc."""
                base = cc * C
                st_tms.pop(cc)
                v_tm = v_tms.pop(cc)
                qt2 = qt2s.pop(cc)
                rbc = rbcs.pop(cc)
                hvb = hvbs[(cc - 1) % 2]
                ets.pop(cc, None)
                pz = psmp.tile([128, 512], F32, tag="pm")
                for vt in range(2):
                    nc.tensor.matmul(pz[:, vt * 256:vt * 256 + 256],
                                     lhsT=v_tm[:, 0, vt * 128:(vt + 1) * 128],
                                     rhs=p2m[:, 0, :], start=True, stop=False)
                    nc.tensor.matmul(pz[:, vt * 256 + 128:vt * 256 + 256],
                                     lhsT=v_tm[:, 1, vt * 128:(vt + 1) * 128],
                                     rhs=p2m[:, 1, 128:256], start=False, stop=False)
                    for mt in range(2):
                        nc.tensor.matmul(pz[:, vt * 256:vt * 256 + 256],
                                         lhsT=hvb[:, mt, vt * 128:(vt + 1) * 128],
                                         rhs=qt2[:, mt, :],
                                         start=False, stop=(mt == 1))
                nc.vector.tensor_tensor(o[:, 0, base:base + C], pz[:, 0:256],
                                        rbc, op=OP.mult)
                nc.vector.tensor_tensor(o[:, 1, base:base + C], pz[:, 256:512],
                                        rbc, op=OP.mult)

            def final_half(th):
                """z = silu(o), z DMA, sq partials for T-half th."""
                t0 = th * 1024
                for vt in range(2):
                    nc.scalar.activation(z[:, vt, t0:t0 + 1024],
                                         o[:, vt, t0:t0 + 1024], AF.Silu)
                    nc.sync.dma_start(out=zv[:, vt, t0:t0 + 1024],
                                      in_=z[:, vt, t0:t0 + 1024])
                pq = [psmp.tile([128, 512], F32, tag="pm", name=f"pq{th}{j}")
                      for j in range(2)]
                for vt in range(2):
                    zsq = wp.tile([128, 1024], BF, tag="zsq", bufs=1)
                    nc.vector.tensor_tensor(zsq, z[:, vt, t0:t0 + 1024],
                                            z[:, vt, t0:t0 + 1024], op=OP.mult)
                    for tq2 in range(2):
                        nc.tensor.matmul(pq[tq2][0:1, :], lhsT=ones_col,
                                         rhs=zsq[:, tq2 * 512:(tq2 + 1) * 512],
                                         start=(vt == 0), stop=(vt == 1))
                for tq2 in range(2):
                    nc.scalar.activation(
                        sqrow[0:1, t0 + tq2 * 512:t0 + (tq2 + 1) * 512],
                        pq[tq2][0:1, :], AF.Copy)

            def sweep_iter(i):
                if 1 <= i <= NCHUNK - 1:
                    qt2_prep(i - 1)
                    hv_stage(i - 1)
                if i + 1 <= NCHUNK - 1:
                    ptm_stage(i + 1)
                if 1 <= i <= NCHUNK - 1:
                    softmax_prep(i - 1)
                p2m = p2m_stage(i - 1) if 1 <= i <= NCHUNK else None
                if 0 <= i <= NCHUNK - 2:
                    hk_stage(i)
                if i + 1 <= NCHUNK - 1:
                    transpose_stage(i + 1)
                if 0 <= i <= NCHUNK - 1:
                    pass1_core(i)
                    if i == NCHUNK - 1:
                        # last chunk: no later iteration, prep early
                        qt2_prep(i)
                        softmax_prep(i)
                if 1 <= i <= NCHUNK:
                    pass2_core(i - 1, p2m)

            SCHED = {0: [-1, 0], 1: [1, 2], 2: [3, 4], 3: [5, 6, 7, 8]}
            for blk in range(4):
                proj_block(blk)
                for i in SCHED[blk]:
                    sweep_iter(i)
                    if i == 4:
                        final_half(0)
            final_half(1)
            nc.sync.dma_start(out=sq_d, in_=sqrow)
    nc.compile()
    return nc


def build_final():
    """Kernel 2: y[t, :] = (z[:, t] * rc[t]) @ wot   for a 512-row t slice.
    zin [1024, 512] bf16 feature-major, wot [1024, 1024] bf16, rc [128, 4] f32.
    Out y [512, 1024] bf16 (row-major (t, d); host upcasts)."""
    nc = bacc.Bacc("TRN2", target_bir_lowering=False, debug=False, num_devices=8)
    z_d = nc.dram_tensor("zin", [D, 512], BF, kind="ExternalInput").ap()
    wo_d = nc.dram_tensor("wot", [D, D], BF, kind="ExternalInput").ap()
    rc_d = nc.dram_tensor("rc", [128, 4], F32, kind="ExternalInput").ap()
    y_d = nc.dram_tensor("y", [512, D], BF, kind="ExternalOutput").ap()

    yv = y_d.rearrange("(a p) d -> p a d", p=128)
    wov = wo_d.rearrange("(a p) o -> p a o", p=128)
    ziv = z_d.rearrange("(a p) t -> p a t", p=128)

    with tile.TileContext(nc) as tc:
        with (
            tc.tile_pool(name="sb", bufs=1) as sb,
            tc.tile_pool(name="yp", bufs=3) as yp,
            tc.tile_pool(name="ps", bufs=4, space="PSUM") as psp,
        ):
            zin = sb.tile([128, 8, 512], BF, tag="z")
            wo = sb.tile([128, 8, 1024], BF, tag="wo")
            rc = sb.tile([128, 4], F32, tag="rc")
            for dc in range(8):   # interleave zin / wot per-dt chunks
                nc.sync.dma_start(out=zin[:, dc:dc + 1, :],
                                  in_=ziv[:, dc:dc + 1, :])
                nc.sync.dma_start(out=wo[:, dc:dc + 1, :],
                                  in_=wov[:, dc:dc + 1, :])
            nc.sync.dma_start(out=rc, in_=rc_d)

            tiles = [psp.tile([128, 1024], F32, tag="ps", name=f"psY{tb}")
                     for tb in range(4)]
            for dt in range(8):
                for tb in range(4):
                    for ot in range(2):
                        nc.tensor.matmul(
                            tiles[tb][:, ot * 512:(ot + 1) * 512],
                            lhsT=zin[:, dt, tb * 128:(tb + 1) * 128],
                            rhs=wo[:, dt, ot * 512:(ot + 1) * 512],
                            start=(dt == 0), stop=(dt == 7))
            for tb in range(4):
                ysb = yp.tile([128, 1024], BF, tag="ysb")
                nc.scalar.activation(ysb[:, 0:512], tiles[tb][:, 0:512],
                                     AF.Copy, scale=rc[:, tb:tb + 1])
                nc.vector.tensor_scalar(
                    out=ysb[:, 512:1024], in0=tiles[tb][:, 512:1024],
                    scalar1=rc[:, tb:tb + 1], scalar2=None, op0=OP.mult)
                nc.sync.dma_start(out=yv[:, tb, :], in_=ysb)
    nc.compile()
    return nc


def _get(name):
    if name not in _cache:
        _cache[name] = build_gsa() if name == "gsa" else build_final()
    return _cache[name]


def kernel(hidden_states, Wq, Wk, Wv, Wf, g_w, Wo, _trace=False):
    bf = ml_dtypes.bfloat16
    hidden_states = np.asarray(hidden_states, np.float32)
    Wq, Wk, Wv, Wf = (np.asarray(x, np.float32) for x in (Wq, Wk, Wv, Wf))
    g_w, Wo = np.asarray(g_w, np.float32), np.asarray(Wo, np.float32)

    mask = np.triu(np.ones((128, 128), np.float32)).astype(bf)  # keep lam <= tau
    ident = np.eye(128).astype(bf)
    ident32 = np.eye(128, dtype=np.float32)
    hst = [np.ascontiguousarray(hidden_states[b].T).astype(bf) for b in range(B)]
    in1 = []
    for core in range(8):
        b, h = core // 4, core % 4
        sl = slice(h * 256, (h + 1) * 256)
        wall = np.concatenate(
            [Wf[sl].T, Wq[sl].T, Wk[sl].T, Wv[sl].T], axis=1)  # [1024, 1024]
        in1.append({
            "hst": hst[b],
            "wall": np.ascontiguousarray(wall).astype(bf),
            "mask": mask,
            "ident": ident,
            "ident32": ident32,
        })
    nc1 = _get("gsa")
    r1 = bass_utils.run_bass_kernel_spmd(nc1, in1, core_ids=list(range(8)),
                                         trace=_trace)
    zs = [r1.results[c]["z"] for c in range(8)]         # each [256, 2048] bf16
    sqs = [r1.results[c]["sq"] for c in range(8)]       # each [1, 2048] f32

    # host: RMS denominators per (b, t)
    rr = []
    for b in range(B):
        ssum = sum(np.asarray(sqs[b * 4 + hh], np.float32) for hh in range(4))[0]
        rr.append(1.0 / np.sqrt(ssum / D + EPS))        # [2048] f32

    wot = np.ascontiguousarray((Wo * g_w[None, :]).T).astype(bf)  # [in, out]
    in2 = []
    for core in range(8):
        b, q = core // 4, core % 4
        zb = np.concatenate([zs[b * 4 + hh] for hh in range(4)], axis=0)
        rc = np.ascontiguousarray(
            rr[b][q * 512:(q + 1) * 512].reshape(4, 128).T).astype(np.float32)
        in2.append({
            "zin": np.ascontiguousarray(zb[:, q * 512:(q + 1) * 512]),
            "wot": wot,
            "rc": rc,
        })
    nc2 = _get("final")
    r2 = bass_utils.run_bass_kernel_spmd(nc2, in2, core_ids=list(range(8)),
                                         trace=_trace)
    out = np.empty((B, T, D), np.float32)
    for core in range(8):
        b, q = core // 4, core % 4
        out[b, q * 512:(q + 1) * 512, :] = np.asarray(
            r2.results[core]["y"], np.float32)
    if _trace:
        kernel.last_traces = (r1, r2)
    return out


# revision 24
# speedup vs baseline: 1.0593x; 1.0593x over previous
"""Gated Slot Attention (GSA) Trainium2 kernel, v4.

Sharding: B*H = 8 lanes -> 8 cores (core = b*4 + h). Kernel 1 ("gsa") does the
per-lane projections + chunked two-pass GLA recurrence, emitting z = silu(o)
feature-major [DV, T] plus the per-lane sum-of-squares row (for RMSNorm).
The host reduces the sq rows across the 4 head-lanes of each batch and
computes the rsqrt denominators; kernel 2 ("final") is then a pure
GEMM + per-row scale with rows of (b,t) split across cores.

Kernel-1 structure:
  Phase A  - weight-stationary projections over the full T=2048: f psums
             (dt-outer, paced by per-dt hs DMA) + e1=exp(-x), then q
             projections, then the gate chain (ln/exp on Act, s/scan/rlam/st
             on DVE) so the psA ring absorbs the Act-serial stretch, then
             k, v projections.  Single fused Silu activation per tile.
  Sweeps   - 8 chunks of C=256 in a pipelined loop: iteration i emits
             softmax-prep for chunk i-1, ptm for chunk i+1, p2m for i-1,
             DMA-engine transposes for i+1, the et/Hk core for i and the
             zt/Hv core for i-1.  PE stages depend only on work emitted
             >= 1 iteration earlier.
             Triangular masking only on the diagonal 128-blocks; the
             strictly-lower block is never computed.
             Hk' = Lend*(Hk + k^T St), Hv' = Lend*(Hv + St^T v) (St-form),
             with the old state folded into PSUM via an identity matmul;
             softmax 1/colsum folded into the final o-multiply.
"""
import sys
sys.path.insert(0, '/opt/trn_rl_repo')

import numpy as np
import ml_dtypes

import concourse.bass as bass
import concourse.bacc as bacc
import concourse.tile as tile
import concourse.mybir as mybir
import concourse.bass_utils as bass_utils

BF = mybir.dt.bfloat16
F32 = mybir.dt.float32
AF = mybir.ActivationFunctionType
OP = mybir.AluOpType

B, T, D = 2, 2048, 1024
H, DK, DV, M = 4, 256, 256, 256
C = 256            # chunk length
NCHUNK = T // C
GATE_NORM = 8.0
EPS = 1e-5

_cache = {}


def build_gsa():
    """Kernel 1: per-lane projections + chunked GLA.
    Outputs zT [256, 2048] bf16 and sq [1, 2048] f32 (sum over dv of z^2)."""
    nc = bacc.Bacc("TRN2", target_bir_lowering=False, debug=False, num_devices=8)
    hsT_d = nc.dram_tensor("hst", [D, T], BF, kind="ExternalInput").ap()
    # wall columns: f 0:256 | q 256:512 | k 512:768 | v 768:1024
    w_d = nc.dram_tensor("wall", [D, 1024], BF, kind="ExternalInput").ap()
    mask_d = nc.dram_tensor("mask", [128, 128], BF, kind="ExternalInput").ap()
    ident_d = nc.dram_tensor("ident", [128, 128], BF, kind="ExternalInput").ap()
    ident32_d = nc.dram_tensor("ident32", [128, 128], F32, kind="ExternalInput").ap()
    z_d = nc.dram_tensor("z", [DV, T], BF, kind="ExternalOutput").ap()
    sq_d = nc.dram_tensor("sq", [1, T], F32, kind="ExternalOutput").ap()

    hsv = hsT_d.rearrange("(a p) t -> p a t", p=128)
    wv = w_d.rearrange("(a p) o -> p a o", p=128)
    zv = z_d.rearrange("(a p) t -> p a t", p=128)

    with tile.TileContext(nc) as tc:
        with (
            tc.tile_pool(name="persist", bufs=1) as pp,
            tc.tile_pool(name="work", bufs=2) as wp,
        ):
            w = pp.tile([128, 8, 1024], BF, tag="w")
            msk = pp.tile([128, 128], BF, tag="msk")
            ident = pp.tile([128, 128], BF, tag="ident")
            ident32 = pp.tile([128, 128], F32, tag="ident32")
            ones_col = pp.tile([128, 1], BF, tag="onescol")
            qT = pp.tile([128, 2, T], BF, tag="qT")
            kT = pp.tile([128, 2, T], BF, tag="kT")
            vT = pp.tile([128, 2, T], BF, tag="vT")
            st = pp.tile([128, 2, T], BF, tag="st")
            lam = pp.tile([128, 2, T], F32, tag="lam")
            o = pp.tile([128, 2, T], BF, tag="o")
            z = pp.tile([128, 2, T], BF, tag="z")
            sqrow = pp.tile([1, T], F32, tag="sqrow")
            hkb = pp.tile([128, 2, 256], BF, tag="hkb")
            hvb = pp.tile([128, 2, 256], BF, tag="hvb")

            # ---------------- Phase A: projections ----------------
            with (
                tc.tile_pool(name="hsp", bufs=1) as hsp,
                tc.tile_pool(name="psA", bufs=4, space="PSUM") as psA,
                tc.tile_pool(name="aw", bufs=2) as aw,
            ):
                hs = hsp.tile([128, 8, T], BF, tag="hs")
                # DMAs: f-weights, then hs by d-tile (paces the f dt-loop),
                # then qkv weights.
                nc.sync.dma_start(out=w[:, :, 0:256], in_=wv[:, :, 0:256])
                for dt in range(8):
                    nc.sync.dma_start(out=hs[:, dt:dt + 1, :],
                                      in_=hsv[:, dt:dt + 1, :])
                nc.sync.dma_start(out=msk, in_=mask_d)
                nc.sync.dma_start(out=ident, in_=ident_d)
                nc.sync.dma_start(out=ident32, in_=ident32_d)
                nc.sync.dma_start(out=w[:, :, 256:1024], in_=wv[:, :, 256:1024])
                nc.vector.memset(ones_col, 1.0)
                nc.vector.memset(hkb, 0.0)
                nc.vector.memset(hvb, 0.0)

                # f projections, dt-outer: all 4 t-quarter psums live at once
                e1 = aw.tile([128, 2, T], BF, tag="aw")
                psfs = [psA.tile([128, 1024], F32, tag="ps", name=f"psf{tq}")
                        for tq in range(4)]
                for dt in range(8):
                    for tq in range(4):
                        t0 = tq * 512
                        for ot in range(2):
                            nc.tensor.matmul(
                                psfs[tq][:, ot * 512:(ot + 1) * 512],
                                lhsT=w[:, dt, ot * 128:(ot + 1) * 128],
                                rhs=hs[:, dt, t0:t0 + 512],
                                start=(dt == 0), stop=(dt == 7))
                for tq in range(4):
                    t0 = tq * 512
                    for ot in range(2):
                        # e1 = exp(-x)
                        nc.scalar.activation(e1[:, ot, t0:t0 + 512],
                                             psfs[tq][:, ot * 512:(ot + 1) * 512],
                                             AF.Exp, scale=-1.0)

                # q, k, v projections (fused Silu), half-T tiles
                def proj(dest, ot, col):
                    for th in range(2):
                        t0 = th * 1024
                        psp = psA.tile([128, 1024], F32, tag="ps", name="psp")
                        for dt in range(8):
                            for tc2 in range(2):
                                nc.tensor.matmul(
                                    psp[:, tc2 * 512:(tc2 + 1) * 512],
                                    lhsT=w[:, dt, col:col + 128],
                                    rhs=hs[:, dt, t0 + tc2 * 512:t0 + (tc2 + 1) * 512],
                                    start=(dt == 0), stop=(dt == 7))
                        for tc2 in range(2):
                            nc.scalar.activation(
                                dest[:, ot, t0 + tc2 * 512:t0 + (tc2 + 1) * 512],
                                psp[:, tc2 * 512:(tc2 + 1) * 512], AF.Silu)
                proj(qT, 0, 256)
                proj(qT, 1, 384)
                # gate chain: Act does ln/exp, DVE does s/scan/rlam/st;
                # emitted between q and k/v so the psA ring absorbs the
                # Act-serial ln/exp stretch.
                nsp = aw.tile([128, 2, T], BF, tag="aw")
                g = aw.tile([128, 2, T], F32, tag="awg", bufs=1)
                s = aw.tile([128, 2, T], BF, tag="aws", bufs=1)
                rlam = aw.tile([128, 2, T], BF, tag="awr", bufs=1)
                for ot in range(2):   # nsp = softplus(-x) = ln(1 + e1)
                    nc.scalar.activation(nsp[:, ot, :], e1[:, ot, :], AF.Ln, bias=1.0)
                for ot in range(2):   # g = exp(-nsp/8)
                    nc.scalar.activation(g[:, ot, :], nsp[:, ot, :], AF.Exp,
                                         scale=-1.0 / GATE_NORM)
                for ot in range(2):   # s = 1 - g   (DVE)
                    nc.vector.tensor_scalar(out=s[:, ot, :], in0=g[:, ot, :],
                                            scalar1=-1.0, scalar2=1.0,
                                            op0=OP.mult, op1=OP.add)
                for ot in range(2):
                    for c in range(NCHUNK):
                        t0 = c * C
                        nc.vector.tensor_tensor_scan(
                            lam[:, ot, t0:t0 + C], g[:, ot, t0:t0 + C],
                            g[:, ot, t0:t0 + C], 1.0, OP.mult, OP.bypass)
                with nc.allow_low_precision(reason="1/lam in bf16 is fine (2e-2 tol)"):
                    for ot in range(2):
                        nc.vector.reciprocal(rlam[:, ot, :], lam[:, ot, :])
                for ot in range(2):
                    nc.vector.tensor_tensor(st[:, ot, :], s[:, ot, :],
                                            rlam[:, ot, :], op=OP.mult)
                proj(kT, 0, 512)
                proj(kT, 1, 640)
                proj(vT, 0, 768)
                proj(vT, 1, 896)

            # ---------------- Sweeps (pipelined) ----------------
            with (
                tc.tile_pool(name="psm", bufs=5, space="PSUM") as psmp,
                tc.tile_pool(name="psq", bufs=2, space="PSUM") as psqp,
            ):
                st_tms, k_tms, v_tms, ptms, ets, bcls, qt2s, rbcs = \
                    {}, {}, {}, {}, {}, {}, {}, {}

                def softmax_prep(cc):
                    """cs, recip, rbc, Qt for chunk cc (all deps >= 1 iter old)."""
                    base = cc * C
                    et = ets[cc]
                    pmisc = psmp.tile([128, 512], F32, tag="pm")
                    for mt in range(2):
                        nc.tensor.matmul(pmisc[0:1, 0:256], lhsT=ones_col,
                                         rhs=et[:, mt, :],
                                         start=(mt == 0), stop=(mt == 1))
                    rrow = wp.tile([1, 256], F32, tag="rrow")
                    nc.vector.reciprocal(rrow, pmisc[0:1, 0:256])
                    rbc = wp.tile([128, 256], F32, tag="rbc")
                    nc.gpsimd.partition_broadcast(rbc, rrow)
                    rbcs[cc] = rbc
                    qt2 = wp.tile([128, 2, 256], BF, tag="qt2")
                    for mt in range(2):
                        nc.gpsimd.tensor_tensor(qt2[:, mt, :], et[:, mt, :],
                                                lam[:, mt, base:base + C],
                                                op=OP.mult)
                    qt2s[cc] = qt2

                def ptm_stage(c):
                    """ptm = mask(k^T q) for chunk c (needs projections only)."""
                    base = c * C
                    pp0 = psmp.tile([128, 512], F32, tag="pm")
                    for k2 in range(2):
                        nc.tensor.matmul(pp0[:, 0:256],
                                         lhsT=kT[:, k2, base:base + 128],
                                         rhs=qT[:, k2, base:base + 256],
                                         start=(k2 == 0), stop=(k2 == 1))
                    for k2 in range(2):
                        nc.tensor.matmul(pp0[:, 256:384],
                                         lhsT=kT[:, k2, base + 128:base + 256],
                                         rhs=qT[:, k2, base + 128:base + 256],
                                         start=(k2 == 0), stop=(k2 == 1))
                    ptm = wp.tile([128, 2, 256], BF, tag="ptm")
                    nc.vector.tensor_tensor(ptm[:, 0, 0:128], pp0[:, 0:128],
                                            msk, op=OP.mult)
                    nc.scalar.activation(ptm[:, 0, 128:256], pp0[:, 128:256], AF.Copy)
                    nc.vector.tensor_tensor(ptm[:, 1, 128:256], pp0[:, 256:384],
                                            msk, op=OP.mult)
                    ptms[c] = ptm

                def p2m_stage(cc):
                    """p2m = mask(St^T Qt) for chunk cc."""
                    base = cc * C
                    qt2 = qt2s[cc]
                    pp2 = psmp.tile([128, 512], F32, tag="pm")
                    for mt in range(2):
                        nc.tensor.matmul(pp2[:, 0:256],
                                         lhsT=st[:, mt, base:base + 128],
                                         rhs=qt2[:, mt, :],
                                         start=(mt == 0), stop=(mt == 1))
                    for mt in range(2):
                        nc.tensor.matmul(pp2[:, 256:384],
                                         lhsT=st[:, mt, base + 128:base + 256],
                                         rhs=qt2[:, mt, 128:256],
                                         start=(mt == 0), stop=(mt == 1))
                    p2m = wp.tile([128, 2, 256], BF, tag="p2m")
                    nc.vector.tensor_tensor(p2m[:, 0, 0:128], pp2[:, 0:128],
                                            msk, op=OP.mult)
                    nc.scalar.activation(p2m[:, 0, 128:256], pp2[:, 128:256], AF.Copy)
                    nc.vector.tensor_tensor(p2m[:, 1, 128:256], pp2[:, 256:384],
                                            msk, op=OP.mult)
                    return p2m

                def transpose_stage(c):
                    """st_tm, k_tm, v_tm via DMA-engine transposes; bcl."""
                    base = c * C
                    st_tm = wp.tile([128, 2, 256], BF, tag="sttm", bufs=3)
                    k_tm = wp.tile([128, 2, 256], BF, tag="ktm")
                    v_tm = wp.tile([128, 2, 256], BF, tag="vtm", bufs=3)
                    for blk in range(2):
                        sl = slice(blk * 128, (blk + 1) * 128)
                        nc.sync.dma_start_transpose(st_tm[:, :, sl],
                                                    st[:, blk, base:base + 256])
                        nc.sync.dma_start_transpose(k_tm[:, :, sl],
                                                    kT[:, blk, base:base + 256])
                        nc.sync.dma_start_transpose(v_tm[:, :, sl],
                                                    vT[:, blk, base:base + 256])
                    st_tms[c], k_tms[c], v_tms[c] = st_tm, k_tm, v_tm

                    # lend row -> bcl broadcast [128, 256] (Lend[m] on free dim)
                    pmisc = psmp.tile([128, 512], F32, tag="pm")
                    for mt in range(2):
                        nc.tensor.transpose(
                            pmisc[0:1, mt * 128:(mt + 1) * 128],
                            lam[:, mt, base + C - 1:base + C], ident32)
                    lrow = wp.tile([1, 256], F32, tag="lrow")
                    nc.vector.tensor_copy(lrow, pmisc[0:1, 0:256])
                    bcl = wp.tile([128, 256], F32, tag="bcl")
                    for mt in range(2):
                        nc.gpsimd.partition_broadcast(
                            bcl[:, mt * 128:(mt + 1) * 128],
                            lrow[0:1, mt * 128:(mt + 1) * 128])
                    bcls[c] = bcl

                def pass1_core(c):
                    """etAB + ok + exp + Hk update for chunk c."""
                    base = c * C
                    st_tm, k_tm = st_tms[c], k_tms.pop(c)
                    ptm = ptms.pop(c)
                    pe0 = psmp.tile([128, 512], F32, tag="pm")
                    for mt in range(2):
                        nc.tensor.matmul(pe0[:, mt * 256:mt * 256 + 256],
                                         lhsT=st_tm[:, 0, mt * 128:(mt + 1) * 128],
                                         rhs=ptm[:, 0, :], start=True, stop=False)
                        nc.tensor.matmul(pe0[:, mt * 256 + 128:mt * 256 + 256],
                                         lhsT=st_tm[:, 1, mt * 128:(mt + 1) * 128],
                                         rhs=ptm[:, 1, 128:256], start=False, stop=False)
                        for k2 in range(2):
                            nc.tensor.matmul(pe0[:, mt * 256:mt * 256 + 256],
                                             lhsT=hkb[:, k2, mt * 128:(mt + 1) * 128],
                                             rhs=qT[:, k2, base:base + 256],
                                             start=False, stop=(k2 == 1))
                    okl = wp.tile([128, 2, 256], F32, tag="okl")
                    nc.vector.tensor_tensor(okl, pe0,
                                            lam[:, :, base:base + C], op=OP.mult)
                    et = wp.tile([128, 2, 256], BF, tag="et")
                    nc.scalar.activation(et, okl, AF.Exp)
                    ets[c] = et

                    # Hk' = Lend * (Hk + k^T St); Hk folded in via ident-matmul
                    bcl = bcls.pop(c)
                    ph = psmp.tile([128, 512], F32, tag="pm")
                    for kb in range(2):
                        for tb in range(2):
                            nc.tensor.matmul(ph[:, kb * 256:kb * 256 + 256],
                                             lhsT=k_tm[:, tb, kb * 128:(kb + 1) * 128],
                                             rhs=st_tm[:, tb, :],
                                             start=(tb == 0), stop=False)
                        nc.tensor.matmul(ph[:, kb * 256:kb * 256 + 256],
                                         lhsT=ident, rhs=hkb[:, kb, :],
                                         start=False, stop=True)
                    for kb in range(2):
                        nc.vector.tensor_tensor(hkb[:, kb, :],
                                                ph[:, kb * 256:kb * 256 + 256],
                                                bcl, op=OP.mult)

                def pass2_core(cc, p2m):
                    """zt + o + Hv update for chunk cc."""
                    base = cc * C
                    st_tm = st_tms.pop(cc)
                    v_tm = v_tms.pop(cc)
                    qt2 = qt2s.pop(cc)
                    rbc = rbcs.pop(cc)
                    ets.pop(cc, None)
                    pz = psmp.tile([128, 512], F32, tag="pm")
                    for vt in range(2):
                        nc.tensor.matmul(pz[:, vt * 256:vt * 256 + 256],
                                         lhsT=v_tm[:, 0, vt * 128:(vt + 1) * 128],
                                         rhs=p2m[:, 0, :], start=True, stop=False)
                        nc.tensor.matmul(pz[:, vt * 256 + 128:vt * 256 + 256],
                                         lhsT=v_tm[:, 1, vt * 128:(vt + 1) * 128],
                                         rhs=p2m[:, 1, 128:256], start=False, stop=False)
                        for mt in range(2):
                            nc.tensor.matmul(pz[:, vt * 256:vt * 256 + 256],
                                             lhsT=hvb[:, mt, vt * 128:(vt + 1) * 128],
                                             rhs=qt2[:, mt, :],
                                             start=False, stop=(mt == 1))
                    nc.vector.tensor_tensor(o[:, 0, base:base + C], pz[:, 0:256],
                                            rbc, op=OP.mult)
                    nc.vector.tensor_tensor(o[:, 1, base:base + C], pz[:, 256:512],
                                            rbc, op=OP.mult)

                    # Hv' = Lend[m] * (Hv + St^T v); Hv folded in via ident-matmul
                    phv = psmp.tile([128, 512], F32, tag="pm")
                    for mt in range(2):
                        for tb in range(2):
                            nc.tensor.matmul(phv[:, mt * 256:mt * 256 + 256],
                                             lhsT=st_tm[:, tb, mt * 128:(mt + 1) * 128],
                                             rhs=v_tm[:, tb, :],
                                             start=(tb == 0), stop=False)
                        nc.tensor.matmul(phv[:, mt * 256:mt * 256 + 256],
                                         lhsT=ident, rhs=hvb[:, mt, :],
                                         start=False, stop=True)
                    for mt in range(2):
                        nc.scalar.activation(hvb[:, mt, :],
                                             phv[:, mt * 256:mt * 256 + 256],
                                             AF.Copy,
                                             scale=lam[:, mt, base + C - 1:base + C])

                for i in range(-1, NCHUNK + 1):
                    if 1 <= i <= NCHUNK - 1:
                        softmax_prep(i - 1)
                    if i + 1 <= NCHUNK - 1:
                        ptm_stage(i + 1)
                    p2m = p2m_stage(i - 1) if 1 <= i <= NCHUNK else None
                    if i + 1 <= NCHUNK - 1:
                        transpose_stage(i + 1)
                    if 0 <= i <= NCHUNK - 1:
                        pass1_core(i)
                        if i == NCHUNK - 1:
                            # last chunk: no later iteration, prep early
                            softmax_prep(i)
                    if 1 <= i <= NCHUNK:
                        pass2_core(i - 1, p2m)

                # ---------------- Final: z = silu(o), sq, DMAs ----------------
                for vt in range(2):
                    nc.scalar.activation(z[:, vt, :], o[:, vt, :], AF.Silu)
                    nc.sync.dma_start(out=zv[:, vt, :], in_=z[:, vt, :])
                for th in range(2):
                    t0 = th * 1024
                    psqt = psqp.tile([1, 2, 512], F32, tag="sq", bufs=1)
                    for vt in range(2):
                        zsq = wp.tile([128, 1024], BF, tag="zsq", bufs=1)
                        nc.vector.tensor_tensor(zsq, z[:, vt, t0:t0 + 1024],
                                                z[:, vt, t0:t0 + 1024], op=OP.mult)
                        for tq2 in range(2):
                            nc.tensor.matmul(psqt[0:1, tq2, :], lhsT=ones_col,
                                             rhs=zsq[:, tq2 * 512:(tq2 + 1) * 512],
                                             start=(vt == 0), stop=(vt == 1))
                    for tq2 in range(2):
                        nc.scalar.activation(
                            sqrow[0:1, t0 + tq2 * 512:t0 + (tq2 + 1) * 512],
                            psqt[0:1, tq2, :], AF.Copy)
                nc.sync.dma_start(out=sq_d, in_=sqrow)
    nc.compile()
    return nc


def build_final():
    """Kernel 2: y[t, :] = (z[:, t] * rc[t]) @ wot   for a 512-row t slice.
    zin [1024, 512] bf16 feature-major, wot [1024, 1024] bf16, rc [128, 4] f32.
    Out y [512, 1024] bf16 (row-major (t, d); host upcasts)."""
    nc = bacc.Bacc("TRN2", target_bir_lowering=False, debug=False, num_devices=8)
    z_d = nc.dram_tensor("zin", [D, 512], BF, kind="ExternalInput").ap()
    wo_d = nc.dram_tensor("wot", [D, D], BF, kind="ExternalInput").ap()
    rc_d = nc.dram_tensor("rc", [128, 4], F32, kind="ExternalInput").ap()
    y_d = nc.dram_tensor("y", [512, D], BF, kind="ExternalOutput").ap()

    yv = y_d.rearrange("(a p) d -> p a d", p=128)
    wov = wo_d.rearrange("(a p) o -> p a o", p=128)
    ziv = z_d.rearrange("(a p) t -> p a t", p=128)

    with tile.TileContext(nc) as tc:
        with (
            tc.tile_pool(name="sb", bufs=1) as sb,
            tc.tile_pool(name="yp", bufs=2) as yp,
            tc.tile_pool(name="ps", bufs=4, space="PSUM") as psp,
        ):
            zin = sb.tile([128, 8, 512], BF, tag="z")
            wo = sb.tile([128, 8, 1024], BF, tag="wo")
            rc = sb.tile([128, 4], F32, tag="rc")
            for dc in range(8):   # interleave zin / wot per-dt chunks
                nc.sync.dma_start(out=zin[:, dc:dc + 1, :],
                                  in_=ziv[:, dc:dc + 1, :])
                nc.sync.dma_start(out=wo[:, dc:dc + 1, :],
                                  in_=wov[:, dc:dc + 1, :])
            nc.sync.dma_start(out=rc, in_=rc_d)

            tiles = [psp.tile([128, 1024], F32, tag="ps", name=f"psY{tb}")
                     for tb in range(4)]
            for dt in range(8):
                for tb in range(4):
                    for ot in range(2):
                        nc.tensor.matmul(
                            tiles[tb][:, ot * 512:(ot + 1) * 512],
                            lhsT=zin[:, dt, tb * 128:(tb + 1) * 128],
                            rhs=wo[:, dt, ot * 512:(ot + 1) * 512],
                            start=(dt == 0), stop=(dt == 7))
            for pair in range(2):
                ysb = yp.tile([128, 2, 1024], BF, tag="ysb", name=f"ysb{pair}")
                for half in range(2):
                    tb = pair * 2 + half
                    nc.scalar.activation(ysb[:, half, 0:512],
                                         tiles[tb][:, 0:512],
                                         AF.Copy, scale=rc[:, tb:tb + 1])
                    nc.vector.tensor_scalar(
                        out=ysb[:, half, 512:1024], in0=tiles[tb][:, 512:1024],
                        scalar1=rc[:, tb:tb + 1], scalar2=None, op0=OP.mult)
                nc.sync.dma_start(out=yv[:, pair * 2:pair * 2 + 2, :], in_=ysb)
    nc.compile()
    return nc


def _get(name):
    if name not in _cache:
        _cache[name] = build_gsa() if name == "gsa" else build_final()
    return _cache[name]


def kernel(hidden_states, Wq, Wk, Wv, Wf, g_w, Wo, _trace=False):
    bf = ml_dtypes.bfloat16
    hidden_states = np.asarray(hidden_states, np.float32)
    Wq, Wk, Wv, Wf = (np.asarray(x, np.float32) for x in (Wq, Wk, Wv, Wf))
    g_w, Wo = np.asarray(g_w, np.float32), np.asarray(Wo, np.float32)

    mask = np.triu(np.ones((128, 128), np.float32)).astype(bf)  # keep lam <= tau
    ident = np.eye(128).astype(bf)
    ident32 = np.eye(128, dtype=np.float32)
    hst = [np.ascontiguousarray(hidden_states[b].T).astype(bf) for b in range(B)]
    in1 = []
    for core in range(8):
        b, h = core // 4, core % 4
        sl = slice(h * 256, (h + 1) * 256)
        wall = np.concatenate(
            [Wf[sl].T, Wq[sl].T, Wk[sl].T, Wv[sl].T], axis=1)  # [1024, 1024]
        in1.append({
            "hst": hst[b],
            "wall": np.ascontiguousarray(wall).astype(bf),
            "mask": mask,
            "ident": ident,
            "ident32": ident32,
        })
    nc1 = _get("gsa")
    r1 = bass_utils.run_bass_kernel_spmd(nc1, in1, core_ids=list(range(8)),
                                         trace=_trace)
    zs = [r1.results[c]["z"] for c in range(8)]         # each [256, 2048] bf16
    sqs = [r1.results[c]["sq"] for c in range(8)]       # each [1, 2048] f32

    # host: RMS denominators per (b, t)
    rr = []
    for b in range(B):
        ssum = sum(np.asarray(sqs[b * 4 + hh], np.float32) for hh in range(4))[0]
        rr.append(1.0 / np.sqrt(ssum / D + EPS))        # [2048] f32

    wot = np.ascontiguousarray((Wo * g_w[None, :]).T).astype(bf)  # [in, out]
    in2 = []
    for core in range(8):
        b, q = core // 4, core % 4
        zb = np.concatenate([zs[b * 4 + hh] for hh in range(4)], axis=0)
        rc = np.ascontiguousarray(
            rr[b][q * 512:(q + 1) * 512].reshape(4, 128).T).astype(np.float32)
        in2.append({
            "zin": np.ascontiguousarray(zb[:, q * 512:(q + 1) * 512]),
            "wot": wot,
            "rc": rc,
        })
    nc2 = _get("final")
    r2 = bass_utils.run_bass_kernel_spmd(nc2, in2, core_ids=list(range(8)),
                                         trace=_trace)
    out = np.empty((B, T, D), np.float32)
    for core in range(8):
        b, q = core // 4, core % 4
        out[b, q * 512:(q + 1) * 512, :] = np.asarray(
            r2.results[core]["y"], np.float32)
    if _trace:
        kernel.last_traces = (r1, r2)
    return out


# revision 25
# speedup vs baseline: 1.0719x; 1.0119x over previous
"""Gated Slot Attention (GSA) Trainium2 kernel, v4.

Sharding: B*H = 8 lanes -> 8 cores (core = b*4 + h). Kernel 1 ("gsa") does the
per-lane projections + chunked two-pass GLA recurrence, emitting z = silu(o)
feature-major [DV, T] plus the per-lane sum-of-squares row (for RMSNorm).
The host reduces the sq rows across the 4 head-lanes of each batch and
computes the rsqrt denominators; kernel 2 ("final") is then a pure
GEMM + per-row scale with rows of (b,t) split across cores.

Kernel-1 structure:
  Phase A  - weight-stationary projections over the full T=2048: f psums
             (dt-outer, paced by per-dt hs DMA) + e1=exp(-x), then q
             projections, then the gate chain (ln/exp on Act, s/scan/rlam/st
             on DVE) so the psA ring absorbs the Act-serial stretch, then
             k, v projections.  Single fused Silu activation per tile.
  Sweeps   - 8 chunks of C=256 in a pipelined loop: iteration i emits
             softmax-prep for chunk i-1, ptm for chunk i+1, p2m for i-1,
             DMA-engine transposes for i+1, the et/Hk core for i and the
             zt/Hv core for i-1.  PE stages depend only on work emitted
             >= 1 iteration earlier.
             Triangular masking only on the diagonal 128-blocks; the
             strictly-lower block is never computed.
             Hk' = Lend*(Hk + k^T St), Hv' = Lend*(Hv + St^T v) (St-form),
             with the old state folded into PSUM via an identity matmul;
             softmax 1/colsum folded into the final o-multiply.
"""
import sys
sys.path.insert(0, '/opt/trn_rl_repo')

import numpy as np
import ml_dtypes

import concourse.bass as bass
import concourse.bacc as bacc
import concourse.tile as tile
import concourse.mybir as mybir
import concourse.bass_utils as bass_utils

BF = mybir.dt.bfloat16
F32 = mybir.dt.float32
AF = mybir.ActivationFunctionType
OP = mybir.AluOpType

B, T, D = 2, 2048, 1024
H, DK, DV, M = 4, 256, 256, 256
C = 256            # chunk length
NCHUNK = T // C
GATE_NORM = 8.0
EPS = 1e-5

_cache = {}


def build_gsa():
    """Kernel 1: per-lane projections + chunked GLA.
    Outputs zT [256, 2048] bf16 and sq [1, 2048] f32 (sum over dv of z^2)."""
    nc = bacc.Bacc("TRN2", target_bir_lowering=False, debug=False, num_devices=8)
    hsT_d = nc.dram_tensor("hst", [D, T], BF, kind="ExternalInput").ap()
    # wall columns: f 0:256 | q 256:512 | k 512:768 | v 768:1024
    w_d = nc.dram_tensor("wall", [D, 1024], BF, kind="ExternalInput").ap()
    mask_d = nc.dram_tensor("mask", [128, 128], BF, kind="ExternalInput").ap()
    ident_d = nc.dram_tensor("ident", [128, 128], BF, kind="ExternalInput").ap()
    ident32_d = nc.dram_tensor("ident32", [128, 128], F32, kind="ExternalInput").ap()
    z_d = nc.dram_tensor("z", [DV, T], BF, kind="ExternalOutput").ap()
    sq_d = nc.dram_tensor("sq", [1, T], F32, kind="ExternalOutput").ap()

    hsv = hsT_d.rearrange("(a p) t -> p a t", p=128)
    wv = w_d.rearrange("(a p) o -> p a o", p=128)
    zv = z_d.rearrange("(a p) t -> p a t", p=128)

    with tile.TileContext(nc) as tc:
        with (
            tc.tile_pool(name="persist", bufs=1) as pp,
            tc.tile_pool(name="work", bufs=2) as wp,
        ):
            w = pp.tile([128, 8, 1024], BF, tag="w")
            msk = pp.tile([128, 128], BF, tag="msk")
            ident = pp.tile([128, 128], BF, tag="ident")
            ident32 = pp.tile([128, 128], F32, tag="ident32")
            ones_col = pp.tile([128, 1], BF, tag="onescol")
            qT = pp.tile([128, 2, T], BF, tag="qT")
            kT = pp.tile([128, 2, T], BF, tag="kT")
            vT = pp.tile([128, 2, T], BF, tag="vT")
            st = pp.tile([128, 2, T], BF, tag="st")
            lam = pp.tile([128, 2, T], F32, tag="lam")
            o = pp.tile([128, 2, T], BF, tag="o")
            z = pp.tile([128, 2, T], BF, tag="z")
            sqrow = pp.tile([1, T], F32, tag="sqrow")
            hkb = pp.tile([128, 2, 256], BF, tag="hkb")
            hvb = pp.tile([128, 2, 256], BF, tag="hvb")

            # ---------------- Phase A: projections ----------------
            with (
                tc.tile_pool(name="hsp", bufs=1) as hsp,
                tc.tile_pool(name="psA", bufs=4, space="PSUM") as psA,
                tc.tile_pool(name="aw", bufs=2) as aw,
            ):
                hs = hsp.tile([128, 8, T], BF, tag="hs")
                # DMAs: f-weights, then hs by d-tile (paces the f dt-loop),
                # then qkv weights.
                nc.sync.dma_start(out=w[:, :, 0:256], in_=wv[:, :, 0:256])
                for dt in range(8):
                    nc.sync.dma_start(out=hs[:, dt:dt + 1, :],
                                      in_=hsv[:, dt:dt + 1, :])
                nc.sync.dma_start(out=msk, in_=mask_d)
                nc.sync.dma_start(out=ident, in_=ident_d)
                nc.sync.dma_start(out=ident32, in_=ident32_d)
                nc.sync.dma_start(out=w[:, :, 256:1024], in_=wv[:, :, 256:1024])
                nc.vector.memset(ones_col, 1.0)
                nc.vector.memset(hkb, 0.0)
                nc.vector.memset(hvb, 0.0)

                # f projections, dt-outer: all 4 t-quarter psums live at once
                e1 = aw.tile([128, 2, T], BF, tag="aw")
                psfs = [psA.tile([128, 1024], F32, tag="ps", name=f"psf{tq}")
                        for tq in range(4)]
                for dt in range(8):
                    for tq in range(4):
                        t0 = tq * 512
                        for ot in range(2):
                            nc.tensor.matmul(
                                psfs[tq][:, ot * 512:(ot + 1) * 512],
                                lhsT=w[:, dt, ot * 128:(ot + 1) * 128],
                                rhs=hs[:, dt, t0:t0 + 512],
                                start=(dt == 0), stop=(dt == 7))
                for tq in range(4):
                    t0 = tq * 512
                    for ot in range(2):
                        # e1 = exp(-x)
                        nc.scalar.activation(e1[:, ot, t0:t0 + 512],
                                             psfs[tq][:, ot * 512:(ot + 1) * 512],
                                             AF.Exp, scale=-1.0)

                # q, k, v projections (fused Silu), half-T tiles
                def proj(dest, ot, col):
                    for th in range(2):
                        t0 = th * 1024
                        psp = psA.tile([128, 1024], F32, tag="ps", name="psp")
                        for dt in range(8):
                            for tc2 in range(2):
                                nc.tensor.matmul(
                                    psp[:, tc2 * 512:(tc2 + 1) * 512],
                                    lhsT=w[:, dt, col:col + 128],
                                    rhs=hs[:, dt, t0 + tc2 * 512:t0 + (tc2 + 1) * 512],
                                    start=(dt == 0), stop=(dt == 7))
                        for tc2 in range(2):
                            nc.scalar.activation(
                                dest[:, ot, t0 + tc2 * 512:t0 + (tc2 + 1) * 512],
                                psp[:, tc2 * 512:(tc2 + 1) * 512], AF.Silu)
                proj(qT, 0, 256)
                proj(qT, 1, 384)
                # gate chain: Act does ln/exp, DVE does s/scan/rlam/st;
                # emitted between q and k/v so the psA ring absorbs the
                # Act-serial ln/exp stretch.
                nsp = aw.tile([128, 2, T], BF, tag="aw")
                g = aw.tile([128, 2, T], F32, tag="awg", bufs=1)
                s = aw.tile([128, 2, T], BF, tag="aws", bufs=1)
                rlam = aw.tile([128, 2, T], BF, tag="awr", bufs=1)
                for ot in range(2):   # nsp = softplus(-x) = ln(1 + e1)
                    nc.scalar.activation(nsp[:, ot, :], e1[:, ot, :], AF.Ln, bias=1.0)
                for ot in range(2):   # g = exp(-nsp/8)
                    nc.scalar.activation(g[:, ot, :], nsp[:, ot, :], AF.Exp,
                                         scale=-1.0 / GATE_NORM)
                for ot in range(2):   # s = 1 - g   (DVE)
                    nc.vector.tensor_scalar(out=s[:, ot, :], in0=g[:, ot, :],
                                            scalar1=-1.0, scalar2=1.0,
                                            op0=OP.mult, op1=OP.add)
                for ot in range(2):
                    for c in range(NCHUNK):
                        t0 = c * C
                        nc.vector.tensor_tensor_scan(
                            lam[:, ot, t0:t0 + C], g[:, ot, t0:t0 + C],
                            g[:, ot, t0:t0 + C], 1.0, OP.mult, OP.bypass)
                with nc.allow_low_precision(reason="1/lam in bf16 is fine (2e-2 tol)"):
                    for ot in range(2):
                        nc.vector.reciprocal(rlam[:, ot, :], lam[:, ot, :])
                for ot in range(2):
                    nc.vector.tensor_tensor(st[:, ot, :], s[:, ot, :],
                                            rlam[:, ot, :], op=OP.mult)
                proj(kT, 0, 512)
                proj(kT, 1, 640)
                proj(vT, 0, 768)
                proj(vT, 1, 896)

            # ---------------- Sweeps (pipelined) ----------------
            with (
                tc.tile_pool(name="psm", bufs=5, space="PSUM") as psmp,
                tc.tile_pool(name="psq", bufs=2, space="PSUM") as psqp,
            ):
                st_tms, k_tms, v_tms, ptms, ets, bcls, qt2s, rbcs = \
                    {}, {}, {}, {}, {}, {}, {}, {}

                def softmax_prep(cc, last=False):
                    """cs, recip, rbc, Qt for chunk cc (all deps >= 1 iter old)."""
                    base = cc * C
                    et = ets[cc]
                    pmisc = psmp.tile([128, 512], F32, tag="pm")
                    for mt in range(2):
                        nc.tensor.matmul(pmisc[0:1, 0:256], lhsT=ones_col,
                                         rhs=et[:, mt, :],
                                         start=(mt == 0), stop=(mt == 1))
                    rrow = wp.tile([1, 256], F32, tag="rrow")
                    nc.vector.reciprocal(rrow, pmisc[0:1, 0:256])
                    rbc = wp.tile([128, 256], F32, tag="rbc")
                    nc.gpsimd.partition_broadcast(rbc, rrow)
                    rbcs[cc] = rbc
                    qt2 = wp.tile([128, 2, 256], BF, tag="qt2")
                    eng = nc.vector if last else nc.gpsimd
                    for mt in range(2):
                        eng.tensor_tensor(qt2[:, mt, :], et[:, mt, :],
                                          lam[:, mt, base:base + C],
                                          op=OP.mult)
                    qt2s[cc] = qt2

                def ptm_stage(c):
                    """ptm = mask(k^T q) for chunk c (needs projections only)."""
                    base = c * C
                    pp0 = psmp.tile([128, 512], F32, tag="pm")
                    for k2 in range(2):
                        nc.tensor.matmul(pp0[:, 0:256],
                                         lhsT=kT[:, k2, base:base + 128],
                                         rhs=qT[:, k2, base:base + 256],
                                         start=(k2 == 0), stop=(k2 == 1))
                    for k2 in range(2):
                        nc.tensor.matmul(pp0[:, 256:384],
                                         lhsT=kT[:, k2, base + 128:base + 256],
                                         rhs=qT[:, k2, base + 128:base + 256],
                                         start=(k2 == 0), stop=(k2 == 1))
                    ptm = wp.tile([128, 2, 256], BF, tag="ptm")
                    nc.vector.tensor_tensor(ptm[:, 0, 0:128], pp0[:, 0:128],
                                            msk, op=OP.mult)
                    nc.scalar.activation(ptm[:, 0, 128:256], pp0[:, 128:256], AF.Copy)
                    nc.vector.tensor_tensor(ptm[:, 1, 128:256], pp0[:, 256:384],
                                            msk, op=OP.mult)
                    ptms[c] = ptm

                def p2m_stage(cc):
                    """p2m = mask(St^T Qt) for chunk cc."""
                    base = cc * C
                    qt2 = qt2s[cc]
                    pp2 = psmp.tile([128, 512], F32, tag="pm")
                    for mt in range(2):
                        nc.tensor.matmul(pp2[:, 0:256],
                                         lhsT=st[:, mt, base:base + 128],
                                         rhs=qt2[:, mt, :],
                                         start=(mt == 0), stop=(mt == 1))
                    for mt in range(2):
                        nc.tensor.matmul(pp2[:, 256:384],
                                         lhsT=st[:, mt, base + 128:base + 256],
                                         rhs=qt2[:, mt, 128:256],
                                         start=(mt == 0), stop=(mt == 1))
                    p2m = wp.tile([128, 2, 256], BF, tag="p2m")
                    nc.vector.tensor_tensor(p2m[:, 0, 0:128], pp2[:, 0:128],
                                            msk, op=OP.mult)
                    nc.scalar.activation(p2m[:, 0, 128:256], pp2[:, 128:256], AF.Copy)
                    nc.vector.tensor_tensor(p2m[:, 1, 128:256], pp2[:, 256:384],
                                            msk, op=OP.mult)
                    return p2m

                def transpose_stage(c):
                    """st_tm, k_tm, v_tm via DMA-engine transposes; bcl."""
                    base = c * C
                    st_tm = wp.tile([128, 2, 256], BF, tag="sttm", bufs=3)
                    k_tm = wp.tile([128, 2, 256], BF, tag="ktm")
                    v_tm = wp.tile([128, 2, 256], BF, tag="vtm", bufs=3)
                    for blk in range(2):
                        sl = slice(blk * 128, (blk + 1) * 128)
                        nc.sync.dma_start_transpose(st_tm[:, :, sl],
                                                    st[:, blk, base:base + 256])
                        nc.sync.dma_start_transpose(k_tm[:, :, sl],
                                                    kT[:, blk, base:base + 256])
                        nc.sync.dma_start_transpose(v_tm[:, :, sl],
                                                    vT[:, blk, base:base + 256])
                    st_tms[c], k_tms[c], v_tms[c] = st_tm, k_tm, v_tm

                    # lend row -> bcl broadcast [128, 256] (Lend[m] on free dim)
                    pmisc = psmp.tile([128, 512], F32, tag="pm")
                    for mt in range(2):
                        nc.tensor.transpose(
                            pmisc[0:1, mt * 128:(mt + 1) * 128],
                            lam[:, mt, base + C - 1:base + C], ident32)
                    lrow = wp.tile([1, 256], F32, tag="lrow")
                    nc.vector.tensor_copy(lrow, pmisc[0:1, 0:256])
                    bcl = wp.tile([128, 256], F32, tag="bcl")
                    for mt in range(2):
                        nc.gpsimd.partition_broadcast(
                            bcl[:, mt * 128:(mt + 1) * 128],
                            lrow[0:1, mt * 128:(mt + 1) * 128])
                    bcls[c] = bcl

                def pass1_core(c):
                    """etAB + ok + exp + Hk update for chunk c."""
                    base = c * C
                    st_tm, k_tm = st_tms[c], k_tms.pop(c)
                    ptm = ptms.pop(c)
                    pe0 = psmp.tile([128, 512], F32, tag="pm")
                    for mt in range(2):
                        nc.tensor.matmul(pe0[:, mt * 256:mt * 256 + 256],
                                         lhsT=st_tm[:, 0, mt * 128:(mt + 1) * 128],
                                         rhs=ptm[:, 0, :], start=True, stop=False)
                        nc.tensor.matmul(pe0[:, mt * 256 + 128:mt * 256 + 256],
                                         lhsT=st_tm[:, 1, mt * 128:(mt + 1) * 128],
                                         rhs=ptm[:, 1, 128:256], start=False, stop=False)
                        for k2 in range(2):
                            nc.tensor.matmul(pe0[:, mt * 256:mt * 256 + 256],
                                             lhsT=hkb[:, k2, mt * 128:(mt + 1) * 128],
                                             rhs=qT[:, k2, base:base + 256],
                                             start=False, stop=(k2 == 1))
                    okl = wp.tile([128, 2, 256], F32, tag="okl")
                    nc.vector.tensor_tensor(okl, pe0,
                                            lam[:, :, base:base + C], op=OP.mult)
                    et = wp.tile([128, 2, 256], BF, tag="et")
                    nc.scalar.activation(et, okl, AF.Exp)
                    ets[c] = et

                    # Hk' = Lend * (Hk + k^T St); Hk folded in via ident-matmul
                    bcl = bcls.pop(c)
                    if c >= NCHUNK - 1:
                        return  # final state never read again
                    ph = psmp.tile([128, 512], F32, tag="pm")
                    for kb in range(2):
                        for tb in range(2):
                            nc.tensor.matmul(ph[:, kb * 256:kb * 256 + 256],
                                             lhsT=k_tm[:, tb, kb * 128:(kb + 1) * 128],
                                             rhs=st_tm[:, tb, :],
                                             start=(tb == 0), stop=False)
                        nc.tensor.matmul(ph[:, kb * 256:kb * 256 + 256],
                                         lhsT=ident, rhs=hkb[:, kb, :],
                                         start=False, stop=True)
                    for kb in range(2):
                        nc.vector.tensor_tensor(hkb[:, kb, :],
                                                ph[:, kb * 256:kb * 256 + 256],
                                                bcl, op=OP.mult)

                def pass2_core(cc, p2m):
                    """zt + o + Hv update for chunk cc."""
                    base = cc * C
                    st_tm = st_tms.pop(cc)
                    v_tm = v_tms.pop(cc)
                    qt2 = qt2s.pop(cc)
                    rbc = rbcs.pop(cc)
                    ets.pop(cc, None)
                    pz = psmp.tile([128, 512], F32, tag="pm")
                    for vt in range(2):
                        nc.tensor.matmul(pz[:, vt * 256:vt * 256 + 256],
                                         lhsT=v_tm[:, 0, vt * 128:(vt + 1) * 128],
                                         rhs=p2m[:, 0, :], start=True, stop=False)
                        nc.tensor.matmul(pz[:, vt * 256 + 128:vt * 256 + 256],
                                         lhsT=v_tm[:, 1, vt * 128:(vt + 1) * 128],
                                         rhs=p2m[:, 1, 128:256], start=False, stop=False)
                        for mt in range(2):
                            nc.tensor.matmul(pz[:, vt * 256:vt * 256 + 256],
                                             lhsT=hvb[:, mt, vt * 128:(vt + 1) * 128],
                                             rhs=qt2[:, mt, :],
                                             start=False, stop=(mt == 1))
                    nc.vector.tensor_tensor(o[:, 0, base:base + C], pz[:, 0:256],
                                            rbc, op=OP.mult)
                    nc.vector.tensor_tensor(o[:, 1, base:base + C], pz[:, 256:512],
                                            rbc, op=OP.mult)
                    if cc >= NCHUNK - 1:
                        return  # final state never read again

                    # Hv' = Lend[m] * (Hv + St^T v); Hv folded in via ident-matmul
                    phv = psmp.tile([128, 512], F32, tag="pm")
                    for mt in range(2):
                        for tb in range(2):
                            nc.tensor.matmul(phv[:, mt * 256:mt * 256 + 256],
                                             lhsT=st_tm[:, tb, mt * 128:(mt + 1) * 128],
                                             rhs=v_tm[:, tb, :],
                                             start=(tb == 0), stop=False)
                        nc.tensor.matmul(phv[:, mt * 256:mt * 256 + 256],
                                         lhsT=ident, rhs=hvb[:, mt, :],
                                         start=False, stop=True)
                    for mt in range(2):
                        nc.scalar.activation(hvb[:, mt, :],
                                             phv[:, mt * 256:mt * 256 + 256],
                                             AF.Copy,
                                             scale=lam[:, mt, base + C - 1:base + C])

                for i in range(-1, NCHUNK + 1):
                    if i + 1 <= NCHUNK - 1:
                        ptm_stage(i + 1)
                    if 1 <= i <= NCHUNK - 1:
                        softmax_prep(i - 1)
                    p2m = p2m_stage(i - 1) if 1 <= i <= NCHUNK else None
                    if i + 1 <= NCHUNK - 1:
                        transpose_stage(i + 1)
                    if 0 <= i <= NCHUNK - 1:
                        pass1_core(i)
                        if i == NCHUNK - 1:
                            # last chunk: no later iteration, prep early
                            softmax_prep(i, last=True)
                    if 1 <= i <= NCHUNK:
                        pass2_core(i - 1, p2m)

                # ---------------- Final: z = silu(o), sq, DMAs ----------------
                for vt in range(2):
                    nc.scalar.activation(z[:, vt, :], o[:, vt, :], AF.Silu)
                    nc.sync.dma_start(out=zv[:, vt, :], in_=z[:, vt, :])
                for th in range(2):
                    t0 = th * 1024
                    psqt = psqp.tile([1, 2, 512], F32, tag="sq", bufs=1)
                    for vt in range(2):
                        zsq = wp.tile([128, 1024], BF, tag="zsq", bufs=1)
                        nc.vector.tensor_tensor(zsq, z[:, vt, t0:t0 + 1024],
                                                z[:, vt, t0:t0 + 1024], op=OP.mult)
                        for tq2 in range(2):
                            nc.tensor.matmul(psqt[0:1, tq2, :], lhsT=ones_col,
                                             rhs=zsq[:, tq2 * 512:(tq2 + 1) * 512],
                                             start=(vt == 0), stop=(vt == 1))
                    for tq2 in range(2):
                        nc.scalar.activation(
                            sqrow[0:1, t0 + tq2 * 512:t0 + (tq2 + 1) * 512],
                            psqt[0:1, tq2, :], AF.Copy)
                nc.sync.dma_start(out=sq_d, in_=sqrow)
    nc.compile()
    return nc


def build_final():
    """Kernel 2: y[t, :] = (z[:, t] * rc[t]) @ wot   for a 512-row t slice.
    zin [1024, 512] bf16 feature-major, wot [1024, 1024] bf16, rc [128, 4] f32.
    Out y [512, 1024] bf16 (row-major (t, d); host upcasts)."""
    nc = bacc.Bacc("TRN2", target_bir_lowering=False, debug=False, num_devices=8)
    z_d = nc.dram_tensor("zin", [D, 512], BF, kind="ExternalInput").ap()
    wo_d = nc.dram_tensor("wot", [D, D], BF, kind="ExternalInput").ap()
    rc_d = nc.dram_tensor("rc", [128, 4], F32, kind="ExternalInput").ap()
    y_d = nc.dram_tensor("y", [512, D], BF, kind="ExternalOutput").ap()

    yv = y_d.rearrange("(a p) d -> p a d", p=128)
    wov = wo_d.rearrange("(a p) o -> p a o", p=128)
    ziv = z_d.rearrange("(a p) t -> p a t", p=128)

    with tile.TileContext(nc) as tc:
        with (
            tc.tile_pool(name="sb", bufs=1) as sb,
            tc.tile_pool(name="yp", bufs=2) as yp,
            tc.tile_pool(name="ps", bufs=4, space="PSUM") as psp,
        ):
            zin = sb.tile([128, 8, 512], BF, tag="z")
            wo = sb.tile([128, 8, 1024], BF, tag="wo")
            rc = sb.tile([128, 4], F32, tag="rc")
            for dc in range(8):   # interleave zin / wot per-dt chunks
                nc.sync.dma_start(out=zin[:, dc:dc + 1, :],
                                  in_=ziv[:, dc:dc + 1, :])
                nc.sync.dma_start(out=wo[:, dc:dc + 1, :],
                                  in_=wov[:, dc:dc + 1, :])
            nc.sync.dma_start(out=rc, in_=rc_d)

            tiles = [psp.tile([128, 1024], F32, tag="ps", name=f"psY{tb}")
                     for tb in range(4)]
            for dt in range(8):
                for tb in range(4):
                    for ot in range(2):
                        nc.tensor.matmul(
                            tiles[tb][:, ot * 512:(ot + 1) * 512],
                            lhsT=zin[:, dt, tb * 128:(tb + 1) * 128],
                            rhs=wo[:, dt, ot * 512:(ot + 1) * 512],
                            start=(dt == 0), stop=(dt == 7))
            for pair in range(2):
                ysb = yp.tile([128, 2, 1024], BF, tag="ysb", name=f"ysb{pair}")
                for half in range(2):
                    tb = pair * 2 + half
                    nc.scalar.activation(ysb[:, half, 0:512],
                                         tiles[tb][:, 0:512],
                                         AF.Copy, scale=rc[:, tb:tb + 1])
                    nc.vector.tensor_scalar(
                        out=ysb[:, half, 512:1024], in0=tiles[tb][:, 512:1024],
                        scalar1=rc[:, tb:tb + 1], scalar2=None, op0=OP.mult)
                nc.sync.dma_start(out=yv[:, pair * 2:pair * 2 + 2, :], in_=ysb)
    nc.compile()
    return nc


def _get(name):
    if name not in _cache:
        _cache[name] = build_gsa() if name == "gsa" else build_final()
    return _cache[name]


def kernel(hidden_states, Wq, Wk, Wv, Wf, g_w, Wo, _trace=False):
    bf = ml_dtypes.bfloat16
    hidden_states = np.asarray(hidden_states, np.float32)
    Wq, Wk, Wv, Wf = (np.asarray(x, np.float32) for x in (Wq, Wk, Wv, Wf))
    g_w, Wo = np.asarray(g_w, np.float32), np.asarray(Wo, np.float32)

    mask = np.triu(np.ones((128, 128), np.float32)).astype(bf)  # keep lam <= tau
    ident = np.eye(128).astype(bf)
    ident32 = np.eye(128, dtype=np.float32)
    hst = [np.ascontiguousarray(hidden_states[b].T).astype(bf) for b in range(B)]
    in1 = []
    for core in range(8):
        b, h = core // 4, core % 4
        sl = slice(h * 256, (h + 1) * 256)
        wall = np.concatenate(
            [Wf[sl].T, Wq[sl].T, Wk[sl].T, Wv[sl].T], axis=1)  # [1024, 1024]
        in1.append({
            "hst": hst[b],
            "wall": np.ascontiguousarray(wall).astype(bf),
            "mask": mask,
            "ident": ident,
            "ident32": ident32,
        })
    nc1 = _get("gsa")
    r1 = bass_utils.run_bass_kernel_spmd(nc1, in1, core_ids=list(range(8)),
                                         trace=_trace)
    zs = [r1.results[c]["z"] for c in range(8)]         # each [256, 2048] bf16
    sqs = [r1.results[c]["sq"] for c in range(8)]       # each [1, 2048] f32

    # host: RMS denominators per (b, t)
    rr = []
    for b in range(B):
        ssum = sum(np.asarray(sqs[b * 4 + hh], np.float32) for hh in range(4))[0]
        rr.append(1.0 / np.sqrt(ssum / D + EPS))        # [2048] f32

    wot = np.ascontiguousarray((Wo * g_w[None, :]).T).astype(bf)  # [in, out]
    in2 = []
    for core in range(8):
        b, q = core // 4, core % 4
        zb = np.concatenate([zs[b * 4 + hh] for hh in range(4)], axis=0)
        rc = np.ascontiguousarray(
            rr[b][q * 512:(q + 1) * 512].reshape(4, 128).T).astype(np.float32)
        in2.append({
            "zin": np.ascontiguousarray(zb[:, q * 512:(q + 1) * 512]),
            "wot": wot,
            "rc": rc,
        })
    nc2 = _get("final")
    r2 = bass_utils.run_bass_kernel_spmd(nc2, in2, core_ids=list(range(8)),
                                         trace=_trace)
    out = np.empty((B, T, D), np.float32)
    for core in range(8):
        b, q = core // 4, core % 4
        out[b, q * 512:(q + 1) * 512, :] = np.asarray(
            r2.results[core]["y"], np.float32)
    if _trace:
        kernel.last_traces = (r1, r2)
    return out


# revision 26
# speedup vs baseline: 1.0915x; 1.0183x over previous
"""Gated Slot Attention (GSA) Trainium2 kernel, v4.

Sharding: B*H = 8 lanes -> 8 cores (core = b*4 + h). Kernel 1 ("gsa") does the
per-lane projections + chunked two-pass GLA recurrence, emitting z = silu(o)
feature-major [DV, T] plus the per-lane sum-of-squares row (for RMSNorm).
The host reduces the sq rows across the 4 head-lanes of each batch and
computes the rsqrt denominators; kernel 2 ("final") is then a pure
GEMM + per-row scale with rows of (b,t) split across cores.

Kernel-1 structure:
  Phase A  - weight-stationary projections over the full T=2048: f psums
             (dt-outer, paced by per-dt hs DMA) + e1=exp(-x), then q
             projections, then the gate chain (ln/exp on Act, s/scan/rlam/st
             on DVE) so the psA ring absorbs the Act-serial stretch, then
             k, v projections.  Single fused Silu activation per tile.
  Sweeps   - 8 chunks of C=256 in a pipelined loop: iteration i emits
             softmax-prep for chunk i-1, ptm for chunk i+1, p2m for i-1,
             DMA-engine transposes for i+1, the et/Hk core for i and the
             zt/Hv core for i-1.  PE stages depend only on work emitted
             >= 1 iteration earlier.
             Triangular masking only on the diagonal 128-blocks; the
             strictly-lower block is never computed.
             Hk' = Lend*(Hk + k^T St), Hv' = Lend*(Hv + St^T v) (St-form),
             with the old state folded into PSUM via an identity matmul;
             softmax 1/colsum folded into the final o-multiply.
"""
import sys
sys.path.insert(0, '/opt/trn_rl_repo')

import numpy as np
import ml_dtypes

import concourse.bass as bass
import concourse.bacc as bacc
import concourse.tile as tile
import concourse.mybir as mybir
import concourse.bass_utils as bass_utils

BF = mybir.dt.bfloat16
F32 = mybir.dt.float32
AF = mybir.ActivationFunctionType
OP = mybir.AluOpType

B, T, D = 2, 2048, 1024
H, DK, DV, M = 4, 256, 256, 256
C = 256            # chunk length
NCHUNK = T // C
GATE_NORM = 8.0
EPS = 1e-5

_cache = {}


def build_gsa():
    """Kernel 1: per-lane projections + chunked GLA.
    Outputs zT [256, 2048] bf16 and sq [1, 2048] f32 (sum over dv of z^2)."""
    nc = bacc.Bacc("TRN2", target_bir_lowering=False, debug=False, num_devices=8)
    hsT_d = nc.dram_tensor("hst", [D, T], BF, kind="ExternalInput").ap()
    # wall columns: f 0:256 | q 256:512 | k 512:768 | v 768:1024
    w_d = nc.dram_tensor("wall", [D, 1024], BF, kind="ExternalInput").ap()
    mask_d = nc.dram_tensor("mask", [128, 128], BF, kind="ExternalInput").ap()
    ident_d = nc.dram_tensor("ident", [128, 128], BF, kind="ExternalInput").ap()
    ident32_d = nc.dram_tensor("ident32", [128, 128], F32, kind="ExternalInput").ap()
    z_d = nc.dram_tensor("z", [DV, T], BF, kind="ExternalOutput").ap()
    sq_d = nc.dram_tensor("sq", [1, T], F32, kind="ExternalOutput").ap()

    hsv = hsT_d.rearrange("(a p) t -> p a t", p=128)
    wv = w_d.rearrange("(a p) o -> p a o", p=128)
    zv = z_d.rearrange("(a p) t -> p a t", p=128)

    with tile.TileContext(nc) as tc:
        with (
            tc.tile_pool(name="persist", bufs=1) as pp,
            tc.tile_pool(name="work", bufs=2) as wp,
        ):
            w = pp.tile([128, 8, 1024], BF, tag="w")
            msk = pp.tile([128, 128], BF, tag="msk")
            ident = pp.tile([128, 128], BF, tag="ident")
            ident32 = pp.tile([128, 128], F32, tag="ident32")
            ones_col = pp.tile([128, 1], BF, tag="onescol")
            qT = pp.tile([128, 2, T], BF, tag="qT")
            kT = pp.tile([128, 2, T], BF, tag="kT")
            vT = pp.tile([128, 2, T], BF, tag="vT")
            st = pp.tile([128, 2, T], BF, tag="st")
            lam = pp.tile([128, 2, T], F32, tag="lam")
            o = pp.tile([128, 2, T], BF, tag="o")
            z = pp.tile([128, 2, T], BF, tag="z")
            sqrow = pp.tile([1, T], F32, tag="sqrow")
            hkb = pp.tile([128, 2, 256], BF, tag="hkb")
            hvb = pp.tile([128, 2, 256], BF, tag="hvb")

            # ---------------- Phase A: projections ----------------
            with (
                tc.tile_pool(name="hsp", bufs=1) as hsp,
                tc.tile_pool(name="psA", bufs=4, space="PSUM") as psA,
                tc.tile_pool(name="aw", bufs=2) as aw,
            ):
                hs = hsp.tile([128, 8, T], BF, tag="hs")
                # DMAs: f-weights, then hs by d-tile (paces the f dt-loop),
                # then qkv weights.
                nc.sync.dma_start(out=w[:, 0:2, 0:256], in_=wv[:, 0:2, 0:256])
                nc.sync.dma_start(out=hs[:, 0:1, :], in_=hsv[:, 0:1, :])
                nc.sync.dma_start(out=w[:, 2:8, 0:256], in_=wv[:, 2:8, 0:256])
                for dt in range(1, 8):
                    nc.sync.dma_start(out=hs[:, dt:dt + 1, :],
                                      in_=hsv[:, dt:dt + 1, :])
                nc.sync.dma_start(out=msk, in_=mask_d)
                nc.sync.dma_start(out=ident, in_=ident_d)
                nc.sync.dma_start(out=ident32, in_=ident32_d)
                nc.sync.dma_start(out=w[:, :, 256:1024], in_=wv[:, :, 256:1024])
                nc.vector.memset(ones_col, 1.0)
                nc.vector.memset(hkb, 0.0)
                nc.vector.memset(hvb, 0.0)

                # f projections, dt-outer: all 4 t-quarter psums live at once
                e1 = aw.tile([128, 2, T], BF, tag="aw")
                psfs = [psA.tile([128, 1024], F32, tag="ps", name=f"psf{tq}")
                        for tq in range(4)]
                for dt in range(8):
                    for tq in range(4):
                        t0 = tq * 512
                        for ot in range(2):
                            nc.tensor.matmul(
                                psfs[tq][:, ot * 512:(ot + 1) * 512],
                                lhsT=w[:, dt, ot * 128:(ot + 1) * 128],
                                rhs=hs[:, dt, t0:t0 + 512],
                                start=(dt == 0), stop=(dt == 7))
                for tq in range(4):
                    t0 = tq * 512
                    for ot in range(2):
                        # e1 = exp(-x)
                        nc.scalar.activation(e1[:, ot, t0:t0 + 512],
                                             psfs[tq][:, ot * 512:(ot + 1) * 512],
                                             AF.Exp, scale=-1.0)

                # q, k, v projections (fused Silu), half-T tiles
                def proj(dest, ot, col):
                    for th in range(2):
                        t0 = th * 1024
                        psp = psA.tile([128, 1024], F32, tag="ps", name="psp")
                        for dt in range(8):
                            for tc2 in range(2):
                                nc.tensor.matmul(
                                    psp[:, tc2 * 512:(tc2 + 1) * 512],
                                    lhsT=w[:, dt, col:col + 128],
                                    rhs=hs[:, dt, t0 + tc2 * 512:t0 + (tc2 + 1) * 512],
                                    start=(dt == 0), stop=(dt == 7))
                        for tc2 in range(2):
                            nc.scalar.activation(
                                dest[:, ot, t0 + tc2 * 512:t0 + (tc2 + 1) * 512],
                                psp[:, tc2 * 512:(tc2 + 1) * 512], AF.Silu)
                proj(qT, 0, 256)
                proj(qT, 1, 384)
                # gate chain: Act does ln/exp, DVE does s/scan/rlam/st;
                # emitted between q and k/v so the psA ring absorbs the
                # Act-serial ln/exp stretch.
                nsp = aw.tile([128, 2, T], BF, tag="aw")
                g = aw.tile([128, 2, T], F32, tag="awg", bufs=1)
                s = aw.tile([128, 2, T], BF, tag="aws", bufs=1)
                rlam = aw.tile([128, 2, T], BF, tag="awr", bufs=1)
                for ot in range(2):   # nsp = softplus(-x) = ln(1 + e1)
                    nc.scalar.activation(nsp[:, ot, :], e1[:, ot, :], AF.Ln, bias=1.0)
                for ot in range(2):   # g = exp(-nsp/8)
                    nc.scalar.activation(g[:, ot, :], nsp[:, ot, :], AF.Exp,
                                         scale=-1.0 / GATE_NORM)
                for ot in range(2):   # s = 1 - g   (DVE)
                    nc.vector.tensor_scalar(out=s[:, ot, :], in0=g[:, ot, :],
                                            scalar1=-1.0, scalar2=1.0,
                                            op0=OP.mult, op1=OP.add)
                for ot in range(2):
                    for c in range(NCHUNK):
                        t0 = c * C
                        nc.vector.tensor_tensor_scan(
                            lam[:, ot, t0:t0 + C], g[:, ot, t0:t0 + C],
                            g[:, ot, t0:t0 + C], 1.0, OP.mult, OP.bypass)
                with nc.allow_low_precision(reason="1/lam in bf16 is fine (2e-2 tol)"):
                    for ot in range(2):
                        nc.vector.reciprocal(rlam[:, ot, :], lam[:, ot, :])
                for ot in range(2):
                    nc.vector.tensor_tensor(st[:, ot, :], s[:, ot, :],
                                            rlam[:, ot, :], op=OP.mult)
                proj(kT, 0, 512)
                proj(kT, 1, 640)
                proj(vT, 0, 768)
                proj(vT, 1, 896)

            # ---------------- Sweeps (pipelined) ----------------
            with (
                tc.tile_pool(name="psm", bufs=5, space="PSUM") as psmp,
                tc.tile_pool(name="psq", bufs=2, space="PSUM") as psqp,
            ):
                st_tms, k_tms, v_tms, ptms, ets, bcls, qt2s, rbcs = \
                    {}, {}, {}, {}, {}, {}, {}, {}

                def softmax_prep(cc, last=False):
                    """cs, recip, rbc, Qt for chunk cc (all deps >= 1 iter old)."""
                    base = cc * C
                    et = ets[cc]
                    pmisc = psmp.tile([128, 512], F32, tag="pm")
                    for mt in range(2):
                        nc.tensor.matmul(pmisc[0:1, 0:256], lhsT=ones_col,
                                         rhs=et[:, mt, :],
                                         start=(mt == 0), stop=(mt == 1))
                    rrow = wp.tile([1, 256], F32, tag="rrow")
                    nc.vector.reciprocal(rrow, pmisc[0:1, 0:256])
                    rbc = wp.tile([128, 256], F32, tag="rbc")
                    nc.gpsimd.partition_broadcast(rbc, rrow)
                    rbcs[cc] = rbc
                    qt2 = wp.tile([128, 2, 256], BF, tag="qt2")
                    eng = nc.vector if last else nc.gpsimd
                    for mt in range(2):
                        eng.tensor_tensor(qt2[:, mt, :], et[:, mt, :],
                                          lam[:, mt, base:base + C],
                                          op=OP.mult)
                    qt2s[cc] = qt2

                def ptm_stage(c):
                    """ptm = mask(k^T q) for chunk c (needs projections only)."""
                    base = c * C
                    pp0 = psmp.tile([128, 512], F32, tag="pm")
                    for k2 in range(2):
                        nc.tensor.matmul(pp0[:, 0:256],
                                         lhsT=kT[:, k2, base:base + 128],
                                         rhs=qT[:, k2, base:base + 256],
                                         start=(k2 == 0), stop=(k2 == 1))
                    for k2 in range(2):
                        nc.tensor.matmul(pp0[:, 256:384],
                                         lhsT=kT[:, k2, base + 128:base + 256],
                                         rhs=qT[:, k2, base + 128:base + 256],
                                         start=(k2 == 0), stop=(k2 == 1))
                    ptm = wp.tile([128, 2, 256], BF, tag="ptm")
                    nc.vector.tensor_tensor(ptm[:, 0, 0:128], pp0[:, 0:128],
                                            msk, op=OP.mult)
                    nc.scalar.activation(ptm[:, 0, 128:256], pp0[:, 128:256], AF.Copy)
                    nc.vector.tensor_tensor(ptm[:, 1, 128:256], pp0[:, 256:384],
                                            msk, op=OP.mult)
                    ptms[c] = ptm

                def p2m_stage(cc):
                    """p2m = mask(St^T Qt) for chunk cc."""
                    base = cc * C
                    qt2 = qt2s[cc]
                    pp2 = psmp.tile([128, 512], F32, tag="pm")
                    for mt in range(2):
                        nc.tensor.matmul(pp2[:, 0:256],
                                         lhsT=st[:, mt, base:base + 128],
                                         rhs=qt2[:, mt, :],
                                         start=(mt == 0), stop=(mt == 1))
                    for mt in range(2):
                        nc.tensor.matmul(pp2[:, 256:384],
                                         lhsT=st[:, mt, base + 128:base + 256],
                                         rhs=qt2[:, mt, 128:256],
                                         start=(mt == 0), stop=(mt == 1))
                    p2m = wp.tile([128, 2, 256], BF, tag="p2m")
                    nc.vector.tensor_tensor(p2m[:, 0, 0:128], pp2[:, 0:128],
                                            msk, op=OP.mult)
                    nc.scalar.activation(p2m[:, 0, 128:256], pp2[:, 128:256], AF.Copy)
                    nc.vector.tensor_tensor(p2m[:, 1, 128:256], pp2[:, 256:384],
                                            msk, op=OP.mult)
                    return p2m

                def transpose_stage(c):
                    """st_tm, k_tm, v_tm via DMA-engine transposes; bcl."""
                    base = c * C
                    st_tm = wp.tile([128, 2, 256], BF, tag="sttm", bufs=3)
                    k_tm = wp.tile([128, 2, 256], BF, tag="ktm")
                    v_tm = wp.tile([128, 2, 256], BF, tag="vtm", bufs=3)
                    for blk in range(2):
                        sl = slice(blk * 128, (blk + 1) * 128)
                        nc.sync.dma_start_transpose(st_tm[:, :, sl],
                                                    st[:, blk, base:base + 256])
                        nc.sync.dma_start_transpose(k_tm[:, :, sl],
                                                    kT[:, blk, base:base + 256])
                        nc.sync.dma_start_transpose(v_tm[:, :, sl],
                                                    vT[:, blk, base:base + 256])
                    st_tms[c], k_tms[c], v_tms[c] = st_tm, k_tm, v_tm

                    # lend row -> bcl broadcast [128, 256] (Lend[m] on free dim)
                    pmisc = psmp.tile([128, 512], F32, tag="pm")
                    for mt in range(2):
                        nc.tensor.transpose(
                            pmisc[0:1, mt * 128:(mt + 1) * 128],
                            lam[:, mt, base + C - 1:base + C], ident32)
                    lrow = wp.tile([1, 256], F32, tag="lrow")
                    nc.vector.tensor_copy(lrow, pmisc[0:1, 0:256])
                    bcl = wp.tile([128, 256], F32, tag="bcl")
                    for mt in range(2):
                        nc.gpsimd.partition_broadcast(
                            bcl[:, mt * 128:(mt + 1) * 128],
                            lrow[0:1, mt * 128:(mt + 1) * 128])
                    bcls[c] = bcl

                def pass1_core(c):
                    """etAB + ok + exp + Hk update for chunk c."""
                    base = c * C
                    st_tm, k_tm = st_tms[c], k_tms.pop(c)
                    ptm = ptms.pop(c)
                    pe0 = psmp.tile([128, 512], F32, tag="pm")
                    for mt in range(2):
                        nc.tensor.matmul(pe0[:, mt * 256:mt * 256 + 256],
                                         lhsT=st_tm[:, 0, mt * 128:(mt + 1) * 128],
                                         rhs=ptm[:, 0, :], start=True, stop=False)
                        nc.tensor.matmul(pe0[:, mt * 256 + 128:mt * 256 + 256],
                                         lhsT=st_tm[:, 1, mt * 128:(mt + 1) * 128],
                                         rhs=ptm[:, 1, 128:256], start=False, stop=False)
                        for k2 in range(2):
                            nc.tensor.matmul(pe0[:, mt * 256:mt * 256 + 256],
                                             lhsT=hkb[:, k2, mt * 128:(mt + 1) * 128],
                                             rhs=qT[:, k2, base:base + 256],
                                             start=False, stop=(k2 == 1))
                    okl = wp.tile([128, 2, 256], F32, tag="okl")
                    nc.vector.tensor_tensor(okl, pe0,
                                            lam[:, :, base:base + C], op=OP.mult)
                    et = wp.tile([128, 2, 256], BF, tag="et")
                    nc.scalar.activation(et, okl, AF.Exp)
                    ets[c] = et

                    # Hk' = Lend * (Hk + k^T St); Hk folded in via ident-matmul
                    bcl = bcls.pop(c)
                    if c >= NCHUNK - 1:
                        return  # final state never read again
                    ph = psmp.tile([128, 512], F32, tag="pm")
                    for kb in range(2):
                        for tb in range(2):
                            nc.tensor.matmul(ph[:, kb * 256:kb * 256 + 256],
                                             lhsT=k_tm[:, tb, kb * 128:(kb + 1) * 128],
                                             rhs=st_tm[:, tb, :],
                                             start=(tb == 0), stop=False)
                        nc.tensor.matmul(ph[:, kb * 256:kb * 256 + 256],
                                         lhsT=ident, rhs=hkb[:, kb, :],
                                         start=False, stop=True)
                    for kb in range(2):
                        nc.vector.tensor_tensor(hkb[:, kb, :],
                                                ph[:, kb * 256:kb * 256 + 256],
                                                bcl, op=OP.mult)

                def pass2_core(cc, p2m):
                    """zt + o + Hv update for chunk cc."""
                    base = cc * C
                    st_tm = st_tms.pop(cc)
                    v_tm = v_tms.pop(cc)
                    qt2 = qt2s.pop(cc)
                    rbc = rbcs.pop(cc)
                    ets.pop(cc, None)
                    pz = psmp.tile([128, 512], F32, tag="pm")
                    for vt in range(2):
                        nc.tensor.matmul(pz[:, vt * 256:vt * 256 + 256],
                                         lhsT=v_tm[:, 0, vt * 128:(vt + 1) * 128],
                                         rhs=p2m[:, 0, :], start=True, stop=False)
                        nc.tensor.matmul(pz[:, vt * 256 + 128:vt * 256 + 256],
                                         lhsT=v_tm[:, 1, vt * 128:(vt + 1) * 128],
                                         rhs=p2m[:, 1, 128:256], start=False, stop=False)
                        for mt in range(2):
                            nc.tensor.matmul(pz[:, vt * 256:vt * 256 + 256],
                                             lhsT=hvb[:, mt, vt * 128:(vt + 1) * 128],
                                             rhs=qt2[:, mt, :],
                                             start=False, stop=(mt == 1))
                    nc.vector.tensor_tensor(o[:, 0, base:base + C], pz[:, 0:256],
                                            rbc, op=OP.mult)
                    nc.vector.tensor_tensor(o[:, 1, base:base + C], pz[:, 256:512],
                                            rbc, op=OP.mult)
                    if cc >= NCHUNK - 1:
                        return  # final state never read again

                    # Hv' = Lend[m] * (Hv + St^T v); Hv folded in via ident-matmul
                    phv = psmp.tile([128, 512], F32, tag="pm")
                    for mt in range(2):
                        for tb in range(2):
                            nc.tensor.matmul(phv[:, mt * 256:mt * 256 + 256],
                                             lhsT=st_tm[:, tb, mt * 128:(mt + 1) * 128],
                                             rhs=v_tm[:, tb, :],
                                             start=(tb == 0), stop=False)
                        nc.tensor.matmul(phv[:, mt * 256:mt * 256 + 256],
                                         lhsT=ident, rhs=hvb[:, mt, :],
                                         start=False, stop=True)
                    for mt in range(2):
                        nc.scalar.activation(hvb[:, mt, :],
                                             phv[:, mt * 256:mt * 256 + 256],
                                             AF.Copy,
                                             scale=lam[:, mt, base + C - 1:base + C])

                for i in range(-1, NCHUNK + 1):
                    if i + 1 <= NCHUNK - 1:
                        ptm_stage(i + 1)
                    if 1 <= i <= NCHUNK - 1:
                        softmax_prep(i - 1)
                    p2m = p2m_stage(i - 1) if 1 <= i <= NCHUNK else None
                    if i + 1 <= NCHUNK - 1:
                        transpose_stage(i + 1)
                    if 0 <= i <= NCHUNK - 1:
                        pass1_core(i)
                        if i == NCHUNK - 1:
                            # last chunk: no later iteration, prep early
                            softmax_prep(i, last=True)
                    if 1 <= i <= NCHUNK:
                        pass2_core(i - 1, p2m)

                # ---------------- Final: z = silu(o), sq, DMAs ----------------
                for vt in range(2):
                    nc.scalar.activation(z[:, vt, :], o[:, vt, :], AF.Silu)
                    nc.sync.dma_start(out=zv[:, vt, :], in_=z[:, vt, :])
                for th in range(2):
                    t0 = th * 1024
                    psqt = psqp.tile([1, 2, 512], F32, tag="sq", bufs=1)
                    for vt in range(2):
                        zsq = wp.tile([128, 1024], BF, tag="zsq", bufs=1)
                        nc.vector.tensor_tensor(zsq, z[:, vt, t0:t0 + 1024],
                                                z[:, vt, t0:t0 + 1024], op=OP.mult)
                        for tq2 in range(2):
                            nc.tensor.matmul(psqt[0:1, tq2, :], lhsT=ones_col,
                                             rhs=zsq[:, tq2 * 512:(tq2 + 1) * 512],
                                             start=(vt == 0), stop=(vt == 1))
                    for tq2 in range(2):
                        nc.scalar.activation(
                            sqrow[0:1, t0 + tq2 * 512:t0 + (tq2 + 1) * 512],
                            psqt[0:1, tq2, :], AF.Copy)
                nc.sync.dma_start(out=sq_d, in_=sqrow)
    nc.compile()
    return nc


def build_final():
    """Kernel 2: y[t, :] = (z[:, t] * rc[t]) @ wot   for a 512-row t slice.
    zin [1024, 512] bf16 feature-major, wot [1024, 1024] bf16, rc [128, 4] f32.
    Out y [512, 1024] bf16 (row-major (t, d); host upcasts)."""
    nc = bacc.Bacc("TRN2", target_bir_lowering=False, debug=False, num_devices=8)
    z_d = nc.dram_tensor("zin", [D, 512], BF, kind="ExternalInput").ap()
    wo_d = nc.dram_tensor("wot", [D, D], BF, kind="ExternalInput").ap()
    rc_d = nc.dram_tensor("rc", [128, 4], F32, kind="ExternalInput").ap()
    y_d = nc.dram_tensor("y", [512, D], BF, kind="ExternalOutput").ap()

    yv = y_d.rearrange("(a p) d -> p a d", p=128)
    wov = wo_d.rearrange("(a p) o -> p a o", p=128)
    ziv = z_d.rearrange("(a p) t -> p a t", p=128)

    with tile.TileContext(nc) as tc:
        with (
            tc.tile_pool(name="sb", bufs=1) as sb,
            tc.tile_pool(name="yp", bufs=2) as yp,
            tc.tile_pool(name="ps", bufs=4, space="PSUM") as psp,
        ):
            zin = sb.tile([128, 8, 512], BF, tag="z")
            wo = sb.tile([128, 8, 1024], BF, tag="wo")
            rc = sb.tile([128, 4], F32, tag="rc")
            for dc in range(8):   # interleave zin / wot per-dt chunks
                nc.sync.dma_start(out=zin[:, dc:dc + 1, :],
                                  in_=ziv[:, dc:dc + 1, :])
                nc.sync.dma_start(out=wo[:, dc:dc + 1, :],
                                  in_=wov[:, dc:dc + 1, :])
            nc.sync.dma_start(out=rc, in_=rc_d)

            tiles = [psp.tile([128, 1024], F32, tag="ps", name=f"psY{tb}")
                     for tb in range(4)]
            for dt in range(8):
                for tb in range(4):
                    for ot in range(2):
                        nc.tensor.matmul(
                            tiles[tb][:, ot * 512:(ot + 1) * 512],
                            lhsT=zin[:, dt, tb * 128:(tb + 1) * 128],
                            rhs=wo[:, dt, ot * 512:(ot + 1) * 512],
                            start=(dt == 0), stop=(dt == 7))
            for pair in range(2):
                ysb = yp.tile([128, 2, 1024], BF, tag="ysb", name=f"ysb{pair}")
                for half in range(2):
                    tb = pair * 2 + half
                    nc.scalar.activation(ysb[:, half, 0:512],
                                         tiles[tb][:, 0:512],
                                         AF.Copy, scale=rc[:, tb:tb + 1])
                    nc.vector.tensor_scalar(
                        out=ysb[:, half, 512:1024], in0=tiles[tb][:, 512:1024],
                        scalar1=rc[:, tb:tb + 1], scalar2=None, op0=OP.mult)
                nc.sync.dma_start(out=yv[:, pair * 2:pair * 2 + 2, :], in_=ysb)
    nc.compile()
    return nc


def _get(name):
    if name not in _cache:
        _cache[name] = build_gsa() if name == "gsa" else build_final()
    return _cache[name]


def kernel(hidden_states, Wq, Wk, Wv, Wf, g_w, Wo, _trace=False):
    bf = ml_dtypes.bfloat16
    hidden_states = np.asarray(hidden_states, np.float32)
    Wq, Wk, Wv, Wf = (np.asarray(x, np.float32) for x in (Wq, Wk, Wv, Wf))
    g_w, Wo = np.asarray(g_w, np.float32), np.asarray(Wo, np.float32)

    mask = np.triu(np.ones((128, 128), np.float32)).astype(bf)  # keep lam <= tau
    ident = np.eye(128).astype(bf)
    ident32 = np.eye(128, dtype=np.float32)
    hst = [np.ascontiguousarray(hidden_states[b].T).astype(bf) for b in range(B)]
    in1 = []
    for core in range(8):
        b, h = core // 4, core % 4
        sl = slice(h * 256, (h + 1) * 256)
        wall = np.concatenate(
            [Wf[sl].T, Wq[sl].T, Wk[sl].T, Wv[sl].T], axis=1)  # [1024, 1024]
        in1.append({
            "hst": hst[b],
            "wall": np.ascontiguousarray(wall).astype(bf),
            "mask": mask,
            "ident": ident,
            "ident32": ident32,
        })
    nc1 = _get("gsa")
    r1 = bass_utils.run_bass_kernel_spmd(nc1, in1, core_ids=list(range(8)),
                                         trace=_trace)
    zs = [r1.results[c]["z"] for c in range(8)]         # each [256, 2048] bf16
    sqs = [r1.results[c]["sq"] for c in range(8)]       # each [1, 2048] f32

    # host: RMS denominators per (b, t)
    rr = []
    for b in range(B):
        ssum = sum(np.asarray(sqs[b * 4 + hh], np.float32) for hh in range(4))[0]
        rr.append(1.0 / np.sqrt(ssum / D + EPS))        # [2048] f32

    wot = np.ascontiguousarray((Wo * g_w[None, :]).T).astype(bf)  # [in, out]
    in2 = []
    for core in range(8):
        b, q = core // 4, core % 4
        zb = np.concatenate([zs[b * 4 + hh] for hh in range(4)], axis=0)
        rc = np.ascontiguousarray(
            rr[b][q * 512:(q + 1) * 512].reshape(4, 128).T).astype(np.float32)
        in2.append({
            "zin": np.ascontiguousarray(zb[:, q * 512:(q + 1) * 512]),
            "wot": wot,
            "rc": rc,
        })
    nc2 = _get("final")
    r2 = bass_utils.run_bass_kernel_spmd(nc2, in2, core_ids=list(range(8)),
                                         trace=_trace)
    out = np.empty((B, T, D), np.float32)
    for core in range(8):
        b, q = core // 4, core % 4
        out[b, q * 512:(q + 1) * 512, :] = np.asarray(
            r2.results[core]["y"], np.float32)
    if _trace:
        kernel.last_traces = (r1, r2)
    return out
